# revision 18
# baseline (speedup 1.0000x reference)
"""Trainium2 Bass kernel for nn_BlockLoRA (GQA attention + LoRA + capacity-routed
top-1 MoE), SPMD over 8 NeuronCores.

Sharding: core c = 2*b + g computes batch b's attention for q-heads
[4g, 4g+4) and kv-head g.  Attention-output head-halves are exchanged
pairwise (AllToAll), after which core c owns global tokens
[1024*c, 1024*(c+1)).  The MoE phase is expert-parallel: core c runs
expert c//2 on capacity slots [1280*c, 1280*(c+1)); dispatch uses an
AllGather of the LN2 output plus an AllToAll'd slot->token inverse map
built with indirect-DMA scatters.
"""

import sys

for _p in ("/opt/trn_rl_repo", "/root/.axon_site/_ro/trn_rl_repo"):
    if _p not in sys.path:
        sys.path.insert(0, _p)

import math

import numpy as np
import ml_dtypes

import concourse.bass as bass
import concourse.bacc as bacc
import concourse.tile as tile
from concourse import mybir
from concourse import bass_utils
from concourse.masks import make_identity

F32 = mybir.dt.float32
F32R = mybir.dt.float32r
BF16 = mybir.dt.bfloat16
I32 = mybir.dt.int32
AX = mybir.AxisListType
OP = mybir.AluOpType
AF = mybir.ActivationFunctionType

C = 384
HQ = 8
HKV = 2
HD = C // HQ          # 48
R = 4
E = 4
T = 2048
B = 4
N_CORES = 8
TH = T // 2           # 1024 tokens per core in phase B
CAP = int(math.ceil(1.25 * B * T / E))   # 2560
HALF = CAP // 2       # 1280 slots per core
F1 = 4 * C            # 1536
SCALE = 1.0 / R
INV_SQRT_HD = 1.0 / math.sqrt(HD)
NTT = T // 128        # 16 token tiles over the full batch
NHT = TH // 128       # 8 token tiles over my half
DUMPED = 999999       # scatter index for dropped tokens
ZROW = N_CORES * TH   # 8192: index of the all-zero row in flat_full
INV_ROWS = (E * CAP // 128 + 1) * 128    # 10368

DEBUG = False


def build(debug=DEBUG):
    nc = bacc.Bacc("TRN2", target_bir_lowering=False, debug=False,
                   num_devices=N_CORES)

    d = {}
    d["x"] = nc.dram_tensor("x", [T, C], F32, kind="ExternalInput")
    d["xh"] = nc.dram_tensor("xh", [TH, C], F32, kind="ExternalInput")
    d["wqkv"] = nc.dram_tensor("wqkv", [C, 288], F32R, kind="ExternalInput")
    d["wo"] = nc.dram_tensor("wo", [C, C], F32R, kind="ExternalInput")
    d["wr"] = nc.dram_tensor("wr", [C, E], F32, kind="ExternalInput")
    d["br"] = nc.dram_tensor("br", [1, E], F32, kind="ExternalInput")
    d["w1"] = nc.dram_tensor("w1", [C, F1], BF16, kind="ExternalInput")
    d["w2"] = nc.dram_tensor("w2", [F1, C], BF16, kind="ExternalInput")
    d["ln1"] = nc.dram_tensor("ln1", [128, 6], F32, kind="ExternalInput")
    d["ln2"] = nc.dram_tensor("ln2", [128, 6], F32, kind="ExternalInput")
    d["ln2gb"] = nc.dram_tensor("ln2gb", [2, C], F32, kind="ExternalInput")
    d["cosq"] = nc.dram_tensor("cosq", [T, 96], F32, kind="ExternalInput")
    d["sinq"] = nc.dram_tensor("sinq", [T, 96], F32, kind="ExternalInput")
    d["cosk"] = nc.dram_tensor("cosk", [T, 24], F32, kind="ExternalInput")
    d["sink"] = nc.dram_tensor("sink", [T, 24], F32, kind="ExternalInput")
    d["wbase"] = nc.dram_tensor("wbase", [8, 1], F32, kind="ExternalInput")
    d["cb"] = nc.dram_tensor("cb", [128, 1], I32, kind="ExternalInput")
    d["orow"] = nc.dram_tensor("orow", [128, 3], I32, kind="ExternalInput")
    d["vmask"] = nc.dram_tensor("vmask", [1, 66], F32R, kind="ExternalInput")
    d["ones48"] = nc.dram_tensor("ones48", [1, 48], F32R, kind="ExternalInput")
    d["out"] = nc.dram_tensor("out", [TH, C], F32, kind="ExternalOutput")
    d["aux"] = nc.dram_tensor("aux", [1, 1], F32, kind="ExternalOutput")

    taps = {}

    def tapf(name, shape, dtype=F32):
        if not debug:
            return None
        taps[name] = nc.dram_tensor("tap_" + name, shape, dtype, kind="ExternalOutput")
        return taps[name]

    d["t_x2"] = tapf("x2", [TH, C])
    d["t_logits"] = tapf("logits", [TH, E])
    d["t_idx"] = tapf("idx", [TH, 1])
    d["t_pos"] = tapf("pos", [TH, 1])
    d["t_gate"] = tapf("gate", [TH, 1])
    d["t_flat"] = tapf("flat", [TH, C])
    d["t_inv"] = tapf("inv", [1, HALF])
    d["t_o2"] = tapf("o2", [HALF, C], BF16)
    d["debug"] = debug

    with tile.TileContext(nc) as tc:
        _body(nc, tc, d)
    nc.compile()
    return nc, taps


def _body(nc, tc, d):
    debug = d["debug"]
    with tc.tile_pool(name="persist", bufs=1) as persist, \
         tc.tile_pool(name="dram", bufs=1, space="DRAM") as dram:

        # ---------------- constants / weights ----------------
        ident = persist.tile([128, 128], F32, name="ident")
        make_identity(nc, ident[:])
        identb = persist.tile([128, 128], BF16, name="identb")
        make_identity(nc, identb[:])
        eps1 = persist.tile([128, 1], F32, name="eps1")
        nc.vector.memset(eps1[:], 1e-5)
        ones48 = persist.tile([1, 48], F32R, name="ones48")
        nc.sync.dma_start(out=ones48[:], in_=d["ones48"][:])
        ones8 = persist.tile([8, 1], F32, name="ones8")
        nc.vector.memset(ones8[:], 1.0)
        ones128r = persist.tile([1, 128], F32, name="ones128r")
        nc.vector.memset(ones128r[:], 1.0)
        ones128c = persist.tile([128, 1], F32, name="ones128c")
        nc.vector.memset(ones128c[:], 1.0)

        # strict upper-triangular ones: triu[j, i] = 1 iff j < i
        triu = persist.tile([128, 128], F32, name="triu")
        nc.gpsimd.memset(triu[:], 1.0)
        nc.gpsimd.affine_select(out=triu[:], in_=triu[:], pattern=[[1, 128]],
                                compare_op=OP.is_gt, fill=0.0, base=0,
                                channel_multiplier=-1)

        easc = persist.tile([128, 4], F32, name="easc")    # 0,1,2,3
        edesc = persist.tile([128, 4], F32, name="edesc")  # 4,3,2,1
        _ei = persist.tile([128, 4], I32, name="_ei")
        nc.gpsimd.iota(out=_ei[:], pattern=[[1, 4]], base=0, channel_multiplier=0)
        nc.vector.tensor_copy(out=easc[:], in_=_ei[:])
        _ei2 = persist.tile([128, 4], I32, name="_ei2")
        nc.gpsimd.iota(out=_ei2[:], pattern=[[-1, 4]], base=4, channel_multiplier=0)
        nc.vector.tensor_copy(out=edesc[:], in_=_ei2[:])

        wqkv_sb = [persist.tile([128, 288], F32R, name=f"wqkv{k}") for k in range(3)]
        wo_sb = [persist.tile([128, C], F32R, name=f"wo{k}") for k in range(3)]
        wr_sb = [persist.tile([128, E], F32, name=f"wr{k}") for k in range(3)]
        w1_sb = [persist.tile([128, F1], BF16, name=f"w1_{k}") for k in range(3)]
        w2_sb = [persist.tile([128, C], BF16, name=f"w2_{k}") for k in range(12)]
        for k in range(3):
            nc.sync.dma_start(out=wqkv_sb[k][:], in_=d["wqkv"][128 * k:128 * (k + 1), :])
            nc.sync.dma_start(out=wo_sb[k][:], in_=d["wo"][128 * k:128 * (k + 1), :])
            nc.sync.dma_start(out=wr_sb[k][:], in_=d["wr"][128 * k:128 * (k + 1), :])
            nc.sync.dma_start(out=w1_sb[k][:], in_=d["w1"][128 * k:128 * (k + 1), :])
        for k in range(12):
            nc.sync.dma_start(out=w2_sb[k][:], in_=d["w2"][128 * k:128 * (k + 1), :])
        br_sb = persist.tile([128, E], F32, name="br_sb")
        nc.sync.dma_start(out=br_sb[:],
                          in_=bass.AP(tensor=d["br"], offset=0, ap=[[0, 128], [1, E]]))
        ln1_sb = persist.tile([128, 6], F32, name="ln1_sb")
        nc.sync.dma_start(out=ln1_sb[:], in_=d["ln1"][:])
        ln2_sb = persist.tile([128, 6], F32, name="ln2_sb")
        nc.sync.dma_start(out=ln2_sb[:], in_=d["ln2"][:])
        g2bc = persist.tile([128, C], F32, name="g2bc")
        b2bc = persist.tile([128, C], F32, name="b2bc")
        nc.sync.dma_start(out=g2bc[:],
                          in_=bass.AP(tensor=d["ln2gb"], offset=0, ap=[[0, 128], [1, C]]))
        nc.sync.dma_start(out=b2bc[:],
                          in_=bass.AP(tensor=d["ln2gb"], offset=C, ap=[[0, 128], [1, C]]))
        wbase_sb = persist.tile([8, 1], F32, name="wbase_sb")
        nc.sync.dma_start(out=wbase_sb[:], in_=d["wbase"][:])
        cb_sb = persist.tile([128, 1], I32, name="cb_sb")
        nc.sync.dma_start(out=cb_sb[:], in_=d["cb"][:])

        # ====================== attention scope ======================
        with tc.tile_pool(name="abuf", bufs=1) as abuf:
            cosq_sb = abuf.tile([128, 96 * NTT], F32, name="cosq_sb")
            sinq_sb = abuf.tile([128, 96 * NTT], F32, name="sinq_sb")
            cosk_sb = abuf.tile([128, 24 * NTT], F32, name="cosk_sb")
            sink_sb = abuf.tile([128, 24 * NTT], F32, name="sink_sb")
            def _tab_ap(dt_, j):
                # sbuf[p, j*t + jj] = dram[128*t + p, jj]
                return bass.AP(tensor=dt_, offset=0,
                               ap=[[j, 128], [128 * j, NTT], [1, j]])
            nc.sync.dma_start(out=cosq_sb[:], in_=_tab_ap(d["cosq"], 96))
            nc.sync.dma_start(out=sinq_sb[:], in_=_tab_ap(d["sinq"], 96))
            nc.sync.dma_start(out=cosk_sb[:], in_=_tab_ap(d["cosk"], 24))
            nc.sync.dma_start(out=sink_sb[:], in_=_tab_ap(d["sink"], 24))

            qT = [abuf.tile([48, T], F32R, name=f"qT{h}") for h in range(4)]
            kT = abuf.tile([48, T], F32R, name="kT")
            v_aug = abuf.tile([128, 66 * NTT], F32R, name="v_aug")
            nc.sync.dma_start(out=v_aug[:],
                              in_=bass.AP(tensor=d["vmask"], offset=0,
                                          ap=[[0, 128], [0, NTT], [1, 66]]))
            oT = [abuf.tile([48, T], F32R, name=f"oT{h}") for h in range(4)]

            # ---- P1-P3: LN1 -> hT -> QKV -> RoPE -> qT/kT/v_aug ----
            with tc.tile_pool(name="p1sb", bufs=3) as p1sb, \
                 tc.tile_pool(name="p1ps", bufs=2, space="PSUM") as p1ps, \
                 tc.tile_pool(name="p1ps2", bufs=2, space="PSUM") as p1ps2:
                for t in range(NTT):
                    xt = p1sb.tile([128, C], F32, name="xt")
                    nc.sync.dma_start(out=xt[:], in_=d["x"][128 * t:128 * (t + 1), :])
                    stats = p1sb.tile([128, 6], F32, name="stats")
                    nc.vector.bn_stats(out=stats[:], in_=xt[:])
                    mv = p1sb.tile([128, 2], F32, name="mv")
                    nc.vector.bn_aggr(out=mv[:], in_=stats[:])
                    rstd = p1sb.tile([128, 1], F32, name="rstd")
                    nc.scalar.activation(out=rstd[:], in_=mv[:, 1:2], func=AF.Sqrt,
                                         bias=eps1[:], scale=1.0)
                    nc.vector.reciprocal(out=rstd[:], in_=rstd[:])
                    xhn = p1sb.tile([128, C], F32, name="xhn")
                    nc.vector.tensor_scalar(out=xhn[:], in0=xt[:], scalar1=mv[:, 0:1],
                                            scalar2=rstd[:], op0=OP.subtract, op1=OP.mult)
                    hTt = []
                    for k in range(3):
                        ptr = p1ps.tile([128, 128], F32, name="ptr")
                        nc.tensor.transpose(out=ptr[:], in_=xhn[:, 128 * k:128 * (k + 1)],
                                            identity=ident[:])
                        hTk = p1sb.tile([128, 128], F32R, name=f"hTk{k}")
                        nc.vector.tensor_scalar(out=hTk[:], in0=ptr[:],
                                                scalar1=ln1_sb[:, k:k + 1],
                                                scalar2=ln1_sb[:, 3 + k:4 + k],
                                                op0=OP.mult, op1=OP.add)
                        hTt.append(hTk)
                    pq = p1ps2.tile([128, 288], F32, name="pq")
                    for k in range(3):
                        nc.tensor.matmul(out=pq[:],
                                         lhsT=hTt[k][:],
                                         rhs=wqkv_sb[k][:],
                                         start=(k == 0), stop=(k == 2))
                    qr = p1sb.tile([128, 192], F32, name="qr")
                    kr = p1sb.tile([128, 48], F32, name="kr")
                    sc1 = p1sb.tile([128, 96], F32, name="sc1")
                    sc2 = p1sb.tile([128, 96], F32, name="sc2")
                    cq = cosq_sb[:, 96 * t:96 * (t + 1)]
                    sq = sinq_sb[:, 96 * t:96 * (t + 1)]
                    ck = cosk_sb[:, 24 * t:24 * (t + 1)]
                    sk = sink_sb[:, 24 * t:24 * (t + 1)]
                    qe, qo = pq[:, 0:192:2], pq[:, 1:192:2]
                    nc.vector.tensor_tensor(out=sc1[:], in0=qe, in1=cq, op=OP.mult)
                    nc.vector.tensor_tensor(out=sc2[:], in0=qo, in1=sq, op=OP.mult)
                    nc.vector.tensor_tensor(out=qr[:, 0:192:2], in0=sc1[:], in1=sc2[:], op=OP.subtract)
                    nc.vector.tensor_tensor(out=sc1[:], in0=qe, in1=sq, op=OP.mult)
                    nc.vector.tensor_tensor(out=sc2[:], in0=qo, in1=cq, op=OP.mult)
                    nc.vector.tensor_tensor(out=qr[:, 1:192:2], in0=sc1[:], in1=sc2[:], op=OP.add)
                    ke, ko = pq[:, 192:240:2], pq[:, 193:240:2]
                    nc.vector.tensor_tensor(out=sc1[:, 0:24], in0=ke, in1=ck, op=OP.mult)
                    nc.vector.tensor_tensor(out=sc2[:, 0:24], in0=ko, in1=sk, op=OP.mult)
                    nc.vector.tensor_tensor(out=kr[:, 0:48:2], in0=sc1[:, 0:24], in1=sc2[:, 0:24], op=OP.subtract)
                    nc.vector.tensor_tensor(out=sc1[:, 0:24], in0=ke, in1=sk, op=OP.mult)
                    nc.vector.tensor_tensor(out=sc2[:, 0:24], in0=ko, in1=ck, op=OP.mult)
                    nc.vector.tensor_tensor(out=kr[:, 1:48:2], in0=sc1[:, 0:24], in1=sc2[:, 0:24], op=OP.add)
                    nc.scalar.activation(out=v_aug[:, 66 * t:66 * t + 48],
                                         in_=pq[:, 240:288], func=AF.Copy, scale=1.0)
                    for h in range(4):
                        ptq = p1ps.tile([48, 128], F32, name="ptq")
                        nc.tensor.transpose(out=ptq[:], in_=qr[:, 48 * h:48 * (h + 1)],
                                            identity=ident[:])
                        nc.vector.tensor_copy(out=qT[h][:, 128 * t:128 * (t + 1)], in_=ptq[:])
                    ptk = p1ps.tile([48, 128], F32, name="ptq")
                    nc.tensor.transpose(out=ptk[:], in_=kr[:], identity=ident[:])
                    nc.vector.tensor_copy(out=kT[:, 128 * t:128 * (t + 1)], in_=ptk[:])

            # ---- P4: attention (windows outer so oT halves finish early) ----
            agg_oT = dram.tile([768, TH], F32R)
            stage_a = dram.tile([192, TH], F32R)
            stage_b = dram.tile([192, TH], F32R)
            with tc.tile_pool(name="atsb", bufs=3) as atsb, \
                 tc.tile_pool(name="atps_s", bufs=2, space="PSUM") as atps_s, \
                 tc.tile_pool(name="atps_o", bufs=2, space="PSUM") as atps_o, \
                 tc.tile_pool(name="atps_b", bufs=2, space="PSUM") as atps_b:
                for w in range(4):
                    for h in range(4):
                        q0 = 512 * w
                        psum_o = atps_o.tile([66, 512], F32, name="psum_o")
                        ngrp = 2 * (w + 1)
                        for grp in range(ngrp):
                            psum_s = atps_s.tile([128, 1024], F32, name="psum_s")
                            pt = atsb.tile([128, 1024], F32R, name="pt")
                            for i in range(2):
                                j = 2 * grp + i
                                nc.tensor.matmul(out=psum_s[:, 512 * i:512 * (i + 1)],
                                                 lhsT=kT[:, 128 * j:128 * (j + 1)],
                                                 rhs=qT[h][:, q0:q0 + 512],
                                                 start=True, stop=True)
                            nc.scalar.activation(out=pt[:], in_=psum_s[:], func=AF.Exp,
                                                 scale=INV_SQRT_HD)
                            for i in range(2):
                                j = 2 * grp + i
                                off = q0 - 128 * j
                                if off < 128:
                                    nc.gpsimd.affine_select(
                                        out=pt[:, 512 * i:512 * (i + 1)],
                                        in_=pt[:, 512 * i:512 * (i + 1)],
                                        pattern=[[1, 512]], compare_op=OP.is_ge,
                                        fill=0.0, base=off, channel_multiplier=-1)
                            for i in range(2):
                                j = 2 * grp + i
                                nc.tensor.matmul(out=psum_o[:],
                                                 lhsT=v_aug[:, 66 * j:66 * j + 66],
                                                 rhs=pt[:, 512 * i:512 * (i + 1)],
                                                 start=(grp == 0 and i == 0),
                                                 stop=(grp == ngrp - 1 and i == 1))
                        rec = atsb.tile([1, 512], F32R, name="rec")
                        with nc.allow_low_precision(reason="f32r softmax denom"):
                            nc.vector.reciprocal(out=rec[:], in_=psum_o[64:65, :])
                        psb = atps_b.tile([48, 512], F32, name="psb")
                        nc.tensor.matmul(out=psb[:], lhsT=ones48[:], rhs=rec[:],
                                         start=True, stop=True)
                        bc = atsb.tile([48, 512], F32, name="bc")
                        nc.vector.tensor_copy(out=bc[:], in_=psb[:])
                        nc.vector.tensor_tensor(out=oT[h][:, q0:q0 + 512],
                                                in0=psum_o[0:48, :], in1=bc[:], op=OP.mult)
                    # after windows 0-1 the first token-half of every head is
                    # done; AllGather it within the batch pair while windows
                    # 2-3 still compute.
                    if w == 1:
                        for h2 in range(4):
                            nc.sync.dma_start(out=stage_a[48 * h2:48 * (h2 + 1), :],
                                              in_=oT[h2][:, 0:TH])
                        nc.gpsimd.collective_compute(
                            "AllGather", OP.bypass,
                            replica_groups=[[0, 1], [2, 3], [4, 5], [6, 7]],
                            ins=[stage_a[:].opt()], outs=[agg_oT[0:384, :].opt()])
                    if w == 3:
                        for h2 in range(4):
                            nc.sync.dma_start(out=stage_b[48 * h2:48 * (h2 + 1), :],
                                              in_=oT[h2][:, TH:T])
                        nc.gpsimd.collective_compute(
                            "AllGather", OP.bypass,
                            replica_groups=[[0, 1], [2, 3], [4, 5], [6, 7]],
                            ins=[stage_b[:].opt()], outs=[agg_oT[384:768, :].opt()])
        # abuf closed

        # ---- P5: o-proj + residual -> x2; LN2 -> flat(+AG) and flatT ----
        # agg_oT rows [384*s + 48*(4*r + h) + dd] = head (4r+h) dim dd of
        # token-half s; my half is s == my pair rank, selected with the
        # per-core orow index vector (indirect gather).
        orow_sb = persist.tile([128, 3], I32, name="orow_sb")
        nc.sync.dma_start(out=orow_sb[:], in_=d["orow"][:])
        oTf = [persist.tile([128, TH], F32R, name=f"oTf{k}") for k in range(3)]
        for k in range(3):
            nc.gpsimd.indirect_dma_start(
                out=oTf[k][:], out_offset=None,
                in_=agg_oT[:],
                in_offset=bass.IndirectOffsetOnAxis(ap=orow_sb[:, k:k + 1], axis=0))
        xh_sb = [persist.tile([128, C], F32, name=f"xh{t}") for t in range(NHT)]
        for t in range(NHT):
            nc.sync.dma_start(out=xh_sb[t][:], in_=d["xh"][128 * t:128 * (t + 1), :])

        x2_sb = [persist.tile([128, C], F32, name=f"x2_{t}") for t in range(NHT)]
        flatT = [persist.tile([128, TH], F32, name=f"flatT{k}") for k in range(3)]
        flat_stage = dram.tile([TH, C], BF16)
        with tc.tile_pool(name="p5sb", bufs=3) as p5sb, \
             tc.tile_pool(name="p5ps", bufs=2, space="PSUM") as p5ps, \
             tc.tile_pool(name="p5ps2", bufs=2, space="PSUM") as p5ps2:
            for t in range(NHT):
                po = p5ps.tile([128, C], F32, name="po")
                for k in range(3):
                    nc.tensor.matmul(out=po[:],
                                     lhsT=oTf[k][:, 128 * t:128 * (t + 1)],
                                     rhs=wo_sb[k][:], start=(k == 0), stop=(k == 2))
                nc.vector.tensor_tensor(out=x2_sb[t][:], in0=po[:], in1=xh_sb[t][:], op=OP.add)
                if debug:
                    nc.sync.dma_start(out=d["t_x2"][128 * t:128 * (t + 1), :], in_=x2_sb[t][:])
                stats = p5sb.tile([128, 6], F32, name="stats")
                nc.vector.bn_stats(out=stats[:], in_=x2_sb[t][:])
                mv = p5sb.tile([128, 2], F32, name="mv")
                nc.vector.bn_aggr(out=mv[:], in_=stats[:])
                rstd = p5sb.tile([128, 1], F32, name="rstd")
                nc.scalar.activation(out=rstd[:], in_=mv[:, 1:2], func=AF.Sqrt,
                                     bias=eps1[:], scale=1.0)
                nc.vector.reciprocal(out=rstd[:], in_=rstd[:])
                xh2 = p5sb.tile([128, C], F32, name="xh2")
                nc.vector.tensor_scalar(out=xh2[:], in0=x2_sb[t][:], scalar1=mv[:, 0:1],
                                        scalar2=rstd[:], op0=OP.subtract, op1=OP.mult)
                fl = p5sb.tile([128, C], F32, name="fl")
                nc.vector.tensor_tensor(out=fl[:], in0=xh2[:], in1=g2bc[:], op=OP.mult)
                nc.vector.tensor_tensor(out=fl[:], in0=fl[:], in1=b2bc[:], op=OP.add)
                flb = p5sb.tile([128, C], BF16, name="flb")
                nc.vector.tensor_copy(out=flb[:], in_=fl[:])
                nc.sync.dma_start(out=flat_stage[128 * t:128 * (t + 1), :], in_=flb[:])
                if debug:
                    nc.sync.dma_start(out=d["t_flat"][128 * t:128 * (t + 1), :], in_=fl[:])
                for k in range(3):
                    ptr = p5ps2.tile([128, 128], F32, name="ptr")
                    nc.tensor.transpose(out=ptr[:], in_=xh2[:, 128 * k:128 * (k + 1)],
                                        identity=ident[:])
                    nc.vector.tensor_scalar(out=flatT[k][:, 128 * t:128 * (t + 1)],
                                            in0=ptr[:],
                                            scalar1=ln2_sb[:, k:k + 1],
                                            scalar2=ln2_sb[:, 3 + k:4 + k],
                                            op0=OP.mult, op1=OP.add)

        # AllGather flat (overlaps router below).  Empty slots carry index
        # ZROW (out of bounds): the gather skips them and the pre-zeroed
        # destination supplies the zero row.
        flat_full = dram.tile([ZROW, C], BF16, addr_space="Shared")
        nc.gpsimd.collective_compute(
            "AllGather", OP.bypass, replica_groups=[list(range(N_CORES))],
            ins=[flat_stage[:].opt()], outs=[flat_full[:].opt()])

        # ---- P7: router; P8: counts AG + aux; P9: positions/slots/scatter;
        #      P10: inverse-map exchange ----
        gate_t = [persist.tile([128, 1], F32, name=f"gate{t}") for t in range(NHT)]
        sclip_t = [persist.tile([128, 1], I32, name=f"sclip{t}") for t in range(NHT)]
        invT = persist.tile([128, HALF // 128], I32, name="invT")
        with tc.tile_pool(name="rtsb", bufs=3) as rtsb, \
             tc.tile_pool(name="rtper", bufs=1) as rtper, \
             tc.tile_pool(name="rtps", bufs=1, space="PSUM") as rtps, \
             tc.tile_pool(name="rtpsc", bufs=1, space="PSUM") as rtpsc:
            idx_t, onehot_t, tv_t, cnt_t = [], [], [], []
            psum_c0 = rtpsc.tile([1, 4], F32, name="psum_c0")
            psum_c1 = rtpsc.tile([1, 4], F32, name="psum_c1")
            for t in range(NHT):
                pl = rtps.tile([128, E], F32, name="pl")
                for k in range(3):
                    nc.tensor.matmul(out=pl[:], lhsT=flatT[k][:, 128 * t:128 * (t + 1)],
                                     rhs=wr_sb[k][:], start=(k == 0), stop=(k == 2))
                lg = rtsb.tile([128, E], F32, name="lg")
                nc.vector.tensor_tensor(out=lg[:], in0=pl[:], in1=br_sb[:], op=OP.add)
                if debug:
                    nc.sync.dma_start(out=d["t_logits"][128 * t:128 * (t + 1), :], in_=lg[:])
                m = rtsb.tile([128, 1], F32, name="m")
                nc.vector.reduce_max(out=m[:], in_=lg[:], axis=AX.X)
                negm = rtsb.tile([128, 1], F32, name="negm")
                nc.vector.tensor_scalar(out=negm[:], in0=m[:], scalar1=-1.0,
                                        scalar2=None, op0=OP.mult)
                pu = rtsb.tile([128, E], F32, name="pu")
                z = rtsb.tile([128, 1], F32, name="z")
                nc.scalar.activation(out=pu[:], in_=lg[:], func=AF.Exp, bias=negm[:],
                                     scale=1.0, accum_out=z[:])
                tv = rtper.tile([128, 1], F32, name=f"tv{t}")
                nc.vector.reciprocal(out=tv[:], in_=z[:])
                probs = rtsb.tile([128, E], F32, name="probs")
                nc.vector.tensor_scalar(out=probs[:], in0=pu[:], scalar1=tv[:],
                                        scalar2=None, op0=OP.mult)
                eq = rtsb.tile([128, E], F32, name="eq")
                nc.vector.tensor_scalar(out=eq[:], in0=lg[:], scalar1=m[:],
                                        scalar2=None, op0=OP.is_ge)
                wt = rtsb.tile([128, E], F32, name="wt")
                nc.vector.tensor_tensor(out=wt[:], in0=eq[:], in1=edesc[:], op=OP.mult)
                rmax = rtsb.tile([128, 1], F32, name="rmax")
                nc.vector.reduce_max(out=rmax[:], in_=wt[:], axis=AX.X)
                idx = rtper.tile([128, 1], F32, name=f"idx{t}")
                nc.vector.tensor_scalar(out=idx[:], in0=rmax[:], scalar1=-1.0,
                                        scalar2=4.0, op0=OP.mult, op1=OP.add)
                oh = rtper.tile([128, E], F32, name=f"oh{t}")
                nc.vector.tensor_tensor(out=oh[:], in0=idx[:].to_broadcast([128, E]),
                                        in1=easc[:], op=OP.is_equal)
                # per-tile expert counts (PE colsum) + running global sums
                pcnt = rtps.tile([1, E], F32, name="pcnt")
                nc.tensor.matmul(out=pcnt[:], lhsT=ones128c[:], rhs=oh[:],
                                 start=True, stop=True)
                cnt = rtper.tile([1, E], F32, name=f"cnt{t}")
                nc.vector.tensor_copy(out=cnt[:], in_=pcnt[:])
                nc.tensor.matmul(out=psum_c0[:], lhsT=ones128c[:], rhs=oh[:],
                                 start=(t == 0), stop=(t == NHT - 1))
                nc.tensor.matmul(out=psum_c1[:], lhsT=ones128c[:], rhs=probs[:],
                                 start=(t == 0), stop=(t == NHT - 1))
                idx_t.append(idx); onehot_t.append(oh); tv_t.append(tv); cnt_t.append(cnt)
                if debug:
                    nc.sync.dma_start(out=d["t_idx"][128 * t:128 * (t + 1), :], in_=idx[:])

            counts_loc = rtper.tile([1, 8], F32, name="counts_loc")
            nc.vector.tensor_copy(out=counts_loc[:, 0:4], in_=psum_c0[:])
            nc.vector.tensor_copy(out=counts_loc[:, 4:8], in_=psum_c1[:])

            # AG#1 counts + prob sums
            ag1_in = dram.tile([1, 8], F32)
            ag1_out = dram.tile([8, 8], F32, addr_space="Shared")
            nc.sync.dma_start(out=ag1_in[:], in_=counts_loc[:])
            nc.gpsimd.collective_compute(
                "AllGather", OP.bypass, replica_groups=[list(range(N_CORES))],
                ins=[ag1_in[:].opt()], outs=[ag1_out[:].opt()])
            ag_sb = rtper.tile([8, 8], F32, name="ag_sb")
            nc.sync.dma_start(out=ag_sb[:], in_=ag1_out[:])
            pbase = rtps.tile([1, E], F32, name="pbase")
            nc.tensor.matmul(out=pbase[:], lhsT=wbase_sb[:], rhs=ag_sb[:, 0:4],
                             start=True, stop=True)
            base_sb = rtper.tile([1, E], F32, name="base_sb")
            nc.vector.tensor_copy(out=base_sb[:], in_=pbase[:])
            psums = rtps.tile([1, 8], F32, name="psums")
            nc.tensor.matmul(out=psums[:], lhsT=ones8[:], rhs=ag_sb[:], start=True, stop=True)
            aux_sb = rtper.tile([1, 1], F32, name="aux_sb")
            cmin = rtper.tile([1, E], F32, name="cmin")
            nc.vector.tensor_scalar(out=cmin[:], in0=psums[:, 0:4], scalar1=float(CAP),
                                    scalar2=None, op0=OP.min)
            smul = rtper.tile([1, E], F32, name="smul")
            nc.vector.tensor_tensor(out=smul[:], in0=cmin[:], in1=psums[:, 4:8], op=OP.mult)
            nc.vector.reduce_sum(out=aux_sb[:], in_=smul[:], axis=AX.X)
            nc.vector.tensor_scalar(out=aux_sb[:], in0=aux_sb[:],
                                    scalar1=float(E) / float(ZROW) ** 2,
                                    scalar2=None, op0=OP.mult)
            nc.sync.dma_start(out=d["aux"][:], in_=aux_sb[:])

            # ---- P9: global positions ----
            inv_local = dram.tile([INV_ROWS, 1], I32)
            zi = rtper.tile([128, INV_ROWS // 128], I32, name="zi")
            nc.vector.memset(zi[:], 0)
            nc.sync.dma_start(
                out=bass.AP(tensor=inv_local.tensor, offset=0,
                            ap=[[INV_ROWS // 128, 128], [1, INV_ROWS // 128]]),
                in_=zi[:])
            r_run = base_sb
            for t in range(NHT):
                ppos = rtps.tile([128, E], F32, name="ppos")
                nc.tensor.matmul(out=ppos[:], lhsT=triu[:], rhs=onehot_t[t][:],
                                 start=True, stop=False)
                nc.tensor.matmul(out=ppos[:], lhsT=ones128r[:], rhs=r_run[:],
                                 start=False, stop=True)
                nr = rtper.tile([1, E], F32, name=f"nr{t}")
                nc.vector.tensor_tensor(out=nr[:], in0=r_run[:], in1=cnt_t[t][:], op=OP.add)
                r_run = nr
                sc = rtsb.tile([128, E], F32, name="sc")
                nc.vector.tensor_tensor(out=sc[:], in0=ppos[:], in1=onehot_t[t][:], op=OP.mult)
                pos = rtsb.tile([128, 1], F32, name="pos")
                nc.vector.reduce_sum(out=pos[:], in_=sc[:], axis=AX.X)
                if debug:
                    nc.sync.dma_start(out=d["t_pos"][128 * t:128 * (t + 1), :], in_=pos[:])
                sbase = rtsb.tile([128, 1], F32, name="sbase")
                nc.vector.tensor_scalar(out=sbase[:], in0=idx_t[t][:], scalar1=float(CAP),
                                        scalar2=None, op0=OP.mult)
                dd_ = rtsb.tile([128, 1], F32, name="dd_")
                nc.vector.tensor_scalar(out=dd_[:], in0=pos[:], scalar1=float(CAP),
                                        scalar2=None, op0=OP.is_lt)
                nc.vector.tensor_tensor(out=gate_t[t][:], in0=tv_t[t][:], in1=dd_[:], op=OP.mult)
                if debug:
                    nc.sync.dma_start(out=d["t_gate"][128 * t:128 * (t + 1), :], in_=gate_t[t][:])
                slot = rtsb.tile([128, 1], F32, name="slot")
                nc.vector.tensor_tensor(out=slot[:], in0=sbase[:], in1=pos[:], op=OP.add)
                se = rtsb.tile([128, 1], F32, name="se")
                nc.vector.tensor_scalar(out=se[:], in0=slot[:], scalar1=float(DUMPED),
                                        scalar2=None, op0=OP.subtract)
                nc.vector.tensor_tensor(out=se[:], in0=se[:], in1=dd_[:], op=OP.mult)
                nc.vector.tensor_scalar(out=se[:], in0=se[:], scalar1=float(DUMPED),
                                        scalar2=None, op0=OP.add)
                sei = rtsb.tile([128, 1], I32, name="sei")
                nc.vector.tensor_copy(out=sei[:], in_=se[:])
                pclip = rtsb.tile([128, 1], F32, name="pclip")
                nc.vector.tensor_scalar(out=pclip[:], in0=pos[:], scalar1=float(CAP - 1),
                                        scalar2=None, op0=OP.min)
                scf = rtsb.tile([128, 1], F32, name="scf")
                nc.vector.tensor_tensor(out=scf[:], in0=sbase[:], in1=pclip[:], op=OP.add)
                nc.vector.tensor_copy(out=sclip_t[t][:], in_=scf[:])
                ti = rtsb.tile([128, 1], I32, name="ti")
                nc.gpsimd.iota(out=ti[:], pattern=[[0, 1]], base=128 * t + 1,
                               channel_multiplier=1)
                tid = rtsb.tile([128, 1], I32, name="tid")
                nc.vector.tensor_tensor(out=tid[:], in0=ti[:], in1=cb_sb[:], op=OP.add)
                nc.gpsimd.indirect_dma_start(
                    out=bass.AP(tensor=inv_local.tensor, offset=0,
                                ap=[[1, E * CAP], [1, 1]]),
                    out_offset=bass.IndirectOffsetOnAxis(ap=sei[:, 0:1], axis=0),
                    in_=tid[:, 0:1], in_offset=None,
                    bounds_check=E * CAP - 1, oob_is_err=False)

            # ---- P10: inverse-map exchange, select my slot range ----
            a2a2_out = dram.tile([8, HALF], I32)
            nc.gpsimd.collective_compute(
                "AllToAll", OP.bypass, replica_groups=[list(range(N_CORES))],
                ins=[bass.AP(tensor=inv_local.tensor, offset=0,
                             ap=[[HALF, 8], [1, HALF]]).opt()],
                outs=[a2a2_out[:].opt()])
            c8i = rtper.tile([8, HALF], I32, name="c8i")
            c8f = rtper.tile([8, HALF], F32, name="c8f")
            nc.sync.dma_start(out=c8i[:], in_=a2a2_out[:])
            nc.vector.tensor_copy(out=c8f[:], in_=c8i[:])
            myinv_f = rtper.tile([1, HALF], F32, name="myinv_f")
            for n in range(0, HALF, 512):
                nn = min(512, HALF - n)
                pinv = rtps.tile([1, 512], F32, name="pinv")
                nc.tensor.matmul(out=pinv[:, 0:nn], lhsT=ones8[:], rhs=c8f[:, n:n + nn],
                                 start=True, stop=True)
                nc.vector.tensor_scalar(out=myinv_f[:, n:n + nn], in0=pinv[:, 0:nn],
                                        scalar1=1.0, scalar2=None, op0=OP.subtract)
            msk = rtper.tile([1, HALF], F32, name="msk")
            nc.vector.tensor_scalar(out=msk[:], in0=myinv_f[:], scalar1=0.0,
                                    scalar2=float(ZROW + 1), op0=OP.is_lt, op1=OP.mult)
            nc.vector.tensor_tensor(out=myinv_f[:], in0=myinv_f[:], in1=msk[:], op=OP.add)
            if debug:
                nc.sync.dma_start(out=d["t_inv"][:], in_=myinv_f[:])
            myinv_i = rtper.tile([1, HALF], I32, name="myinv_i")
            nc.vector.tensor_copy(out=myinv_i[:], in_=myinv_f[:])
            # SBUF partitions are physical: bounce through DRAM to re-partition
            inv_dram = dram.tile([1, HALF], I32)
            nc.sync.dma_start(out=inv_dram[:], in_=myinv_i[:])
            nc.sync.dma_start(out=invT[:],
                              in_=bass.AP(tensor=inv_dram.tensor, offset=0,
                                          ap=[[1, 128], [128, HALF // 128]]))

        # ---- P11: expert FFN over my 1280 slots ----
        o2_stage = dram.tile([HALF, C], BF16)
        with tc.tile_pool(name="mosb", bufs=3) as mosb, \
             tc.tile_pool(name="moper", bufs=1) as moper, \
             tc.tile_pool(name="mops", bufs=2, space="PSUM") as mops, \
             tc.tile_pool(name="mops2", bufs=2, space="PSUM") as mops2:
            bufT = [moper.tile([128, HALF], BF16, name=f"bufT{k}") for k in range(3)]
            for kt in range(HALF // 128):
                gb = mosb.tile([128, C], BF16, name="gb")
                nc.vector.memset(gb[:], 0.0)
                nc.gpsimd.indirect_dma_start(
                    out=gb[:], out_offset=None,
                    in_=flat_full[:],
                    in_offset=bass.IndirectOffsetOnAxis(ap=invT[:, kt:kt + 1], axis=0),
                    bounds_check=ZROW - 1, oob_is_err=False)
                for k in range(3):
                    ptr = mops2.tile([128, 128], BF16, name="ptr")
                    nc.tensor.transpose(out=ptr[:], in_=gb[:, 128 * k:128 * (k + 1)],
                                        identity=identb[:])
                    nc.vector.tensor_copy(out=bufT[k][:, 128 * kt:128 * (kt + 1)], in_=ptr[:])
            h1T = [moper.tile([128, HALF], BF16, name=f"h1T{m}") for m in range(12)]
            nwin = [(0, 512), (512, 1024), (1024, 1280)]
            for m in range(12):
                ph = mops.tile([128, HALF], F32, name="ph")
                for (n0, n1) in nwin:
                    for k in range(3):
                        nc.tensor.matmul(out=ph[:, n0:n1],
                                         lhsT=w1_sb[k][:, 128 * m:128 * (m + 1)],
                                         rhs=bufT[k][:, n0:n1],
                                         start=(k == 0), stop=(k == 2))
                nc.scalar.activation(out=h1T[m][:], in_=ph[:], func=AF.Relu, scale=1.0)
            o2T = [moper.tile([128, HALF], BF16, name=f"o2T{m}") for m in range(3)]
            for m in range(3):
                ph = mops.tile([128, HALF], F32, name="ph")
                for (n0, n1) in nwin:
                    for k in range(12):
                        nc.tensor.matmul(out=ph[:, n0:n1],
                                         lhsT=w2_sb[k][:, 128 * m:128 * (m + 1)],
                                         rhs=h1T[k][:, n0:n1],
                                         start=(k == 0), stop=(k == 11))
                nc.vector.tensor_copy(out=o2T[m][:], in_=ph[:])
            for kt in range(HALF // 128):
                ob = mosb.tile([128, C], BF16, name="ob")
                for m in range(3):
                    ptr = mops2.tile([128, 128], BF16, name="ptr")
                    nc.tensor.transpose(out=ptr[:], in_=o2T[m][:, 128 * kt:128 * (kt + 1)],
                                        identity=identb[:])
                    nc.vector.tensor_copy(out=ob[:, 128 * m:128 * (m + 1)], in_=ptr[:])
                nc.sync.dma_start(out=o2_stage[128 * kt:128 * (kt + 1), :], in_=ob[:])
                if debug:
                    nc.sync.dma_start(out=d["t_o2"][128 * kt:128 * (kt + 1), :], in_=ob[:])

        o2_full = dram.tile([E * CAP, C], BF16, addr_space="Shared")
        nc.gpsimd.collective_compute(
            "AllGather", OP.bypass, replica_groups=[list(range(N_CORES))],
            ins=[o2_stage[:].opt()], outs=[o2_full[:].opt()])

        # ---- P12: final gather + gate + residual ----
        with tc.tile_pool(name="fsb", bufs=3) as fsb:
            for t in range(NHT):
                og = fsb.tile([128, C], BF16, name="og")
                nc.gpsimd.indirect_dma_start(
                    out=og[:], out_offset=None,
                    in_=o2_full[:],
                    in_offset=bass.IndirectOffsetOnAxis(ap=sclip_t[t][:, 0:1], axis=0))
                sg = fsb.tile([128, C], F32, name="sg")
                nc.vector.tensor_scalar(out=sg[:], in0=og[:], scalar1=gate_t[t][:],
                                        scalar2=None, op0=OP.mult)
                ot = fsb.tile([128, C], F32, name="ot")
                nc.vector.tensor_tensor(out=ot[:], in0=sg[:], in1=x2_sb[t][:], op=OP.add)
                nc.sync.dma_start(out=d["out"][128 * t:128 * (t + 1), :], in_=ot[:])


# ---------------------------------------------------------------------------
# Host side
# ---------------------------------------------------------------------------

def _fold(W, A, B_):
    return (np.asarray(W, np.float64)
            + SCALE * (np.asarray(A, np.float64).T @ np.asarray(B_, np.float64).T)
            ).astype(np.float32)


def _rope_tables():
    inv = 1.0 / (10000.0 ** (np.arange(0, HD, 2, dtype=np.float64) / HD))
    ang = np.arange(T, dtype=np.float64)[:, None] * inv
    cos = np.cos(ang).astype(np.float32)
    sin = np.sin(ang).astype(np.float32)
    return np.tile(cos, (1, 4)), np.tile(sin, (1, 4)), cos, sin


def _vmask():
    v = np.zeros((1, 66), np.float32)
    v[0, 64] = 1.0
    return v


def make_in_maps(inputs):
    x = np.asarray(inputs["x"], np.float32)
    aid = int(np.asarray(inputs["adapter_id"]))
    Wq = _fold(inputs["Wq"], np.asarray(inputs["Aq"])[aid], np.asarray(inputs["Bq"])[aid])
    Wk = _fold(inputs["Wk"], np.asarray(inputs["Ak"])[aid], np.asarray(inputs["Bk"])[aid])
    Wv = _fold(inputs["Wv"], np.asarray(inputs["Av"])[aid], np.asarray(inputs["Bv"])[aid])
    Wo = _fold(inputs["Wo"], np.asarray(inputs["Ao"])[aid], np.asarray(inputs["Bo"])[aid])
    Wr = np.ascontiguousarray(np.asarray(inputs["Wr"], np.float32))
    br = np.asarray(inputs["br"], np.float32).reshape(1, E)
    W1 = [_fold(np.asarray(inputs["W1"])[e], np.asarray(inputs["A1"])[e, aid],
                np.asarray(inputs["B1"])[e, aid]) for e in range(E)]
    W2 = [_fold(np.asarray(inputs["W2"])[e], np.asarray(inputs["A2"])[e, aid],
                np.asarray(inputs["B2"])[e, aid]) for e in range(E)]
    ln1g = np.asarray(inputs["ln1_g"], np.float32)
    ln1b = np.asarray(inputs["ln1_b"], np.float32)
    ln2g = np.asarray(inputs["ln2_g"], np.float32)
    ln2b = np.asarray(inputs["ln2_b"], np.float32)
    ln1 = np.concatenate([ln1g.reshape(3, 128).T, ln1b.reshape(3, 128).T], 1)
    ln2 = np.concatenate([ln2g.reshape(3, 128).T, ln2b.reshape(3, 128).T], 1)
    ln2gb = np.stack([ln2g, ln2b])
    cosq, sinq, cosk, sink = _rope_tables()

    in_maps = []
    for c in range(N_CORES):
        b, g = c // 2, c % 2
        e = c // 2
        wqkv = np.concatenate([
            Wq[:, 192 * g:192 * (g + 1)],
            Wk[:, 48 * g:48 * (g + 1)],
            Wv[:, 48 * g:48 * (g + 1)],
        ], axis=1)
        in_maps.append({
            "x": np.ascontiguousarray(x[b]),
            "xh": np.ascontiguousarray(x[b, TH * g:TH * (g + 1)]),
            "wqkv": np.ascontiguousarray(wqkv),
            "wo": Wo,
            "wr": Wr,
            "br": br,
            "w1": W1[e].astype(ml_dtypes.bfloat16),
            "w2": W2[e].astype(ml_dtypes.bfloat16),
            "ln1": np.ascontiguousarray(ln1),
            "ln2": np.ascontiguousarray(ln2),
            "ln2gb": np.ascontiguousarray(ln2gb),
            "cosq": np.ascontiguousarray(cosq),
            "sinq": np.ascontiguousarray(sinq),
            "cosk": np.ascontiguousarray(cosk),
            "sink": np.ascontiguousarray(sink),
            "wbase": (np.arange(8) < c).astype(np.float32).reshape(8, 1),
            "vmask": _vmask(),
            "ones48": np.ones((1, 48), np.float32),
            "cb": np.full((128, 1), TH * c, np.int32),
            "orow": (384 * g + 128 * np.arange(3)[None, :]
                     + np.arange(128)[:, None]).astype(np.int32),
        })
    return in_maps


_CACHED = {}


def _get_nc(debug=False):
    key = bool(debug)
    if key not in _CACHED:
        _CACHED[key] = build(key)
    return _CACHED[key]


def assemble(results):
    full = np.concatenate([results[c]["out"] for c in range(N_CORES)], 0)
    out = full.reshape(B, T, C)
    aux = np.float32(results[0]["aux"][0, 0])
    return out, aux


def kernel(**inputs):
    nc, _ = _get_nc(False)
    in_maps = make_in_maps(inputs)
    res = bass_utils.run_bass_kernel_spmd(nc, in_maps, core_ids=list(range(N_CORES)))
    return assemble(res.results)


# revision 20
# speedup vs baseline: 1.0069x; 1.0069x over previous
"""Trainium2 Bass kernel for nn_BlockLoRA (GQA attention + LoRA + capacity-routed
top-1 MoE), SPMD over 8 NeuronCores.

Sharding: core c = 2*b + g computes batch b's attention for q-heads
[4g, 4g+4) and kv-head g.  Attention-output head-halves are exchanged
pairwise (AllToAll), after which core c owns global tokens
[1024*c, 1024*(c+1)).  The MoE phase is expert-parallel: core c runs
expert c//2 on capacity slots [1280*c, 1280*(c+1)); dispatch uses an
AllGather of the LN2 output plus an AllToAll'd slot->token inverse map
built with indirect-DMA scatters.
"""

import sys

for _p in ("/opt/trn_rl_repo", "/root/.axon_site/_ro/trn_rl_repo"):
    if _p not in sys.path:
        sys.path.insert(0, _p)

import math

import numpy as np
import ml_dtypes

import concourse.bass as bass
import concourse.bacc as bacc
import concourse.tile as tile
from concourse import mybir
from concourse import bass_utils
from concourse.masks import make_identity

F32 = mybir.dt.float32
F32R = mybir.dt.float32r
BF16 = mybir.dt.bfloat16
I32 = mybir.dt.int32
AX = mybir.AxisListType
OP = mybir.AluOpType
AF = mybir.ActivationFunctionType

C = 384
HQ = 8
HKV = 2
HD = C // HQ          # 48
R = 4
E = 4
T = 2048
B = 4
N_CORES = 8
TH = T // 2           # 1024 tokens per core in phase B
CAP = int(math.ceil(1.25 * B * T / E))   # 2560
HALF = CAP // 2       # 1280 slots per core
F1 = 4 * C            # 1536
SCALE = 1.0 / R
INV_SQRT_HD = 1.0 / math.sqrt(HD)
NTT = T // 128        # 16 token tiles over the full batch
NHT = TH // 128       # 8 token tiles over my half
DUMPED = 999999       # scatter index for dropped tokens
ZROW = N_CORES * TH   # 8192: index of the all-zero row in flat_full
INV_ROWS = (E * CAP // 128 + 1) * 128    # 10368

DEBUG = False


def build(debug=DEBUG):
    nc = bacc.Bacc("TRN2", target_bir_lowering=False, debug=False,
                   num_devices=N_CORES)

    d = {}
    d["x"] = nc.dram_tensor("x", [T, C], F32, kind="ExternalInput")
    d["xh"] = nc.dram_tensor("xh", [TH, C], F32, kind="ExternalInput")
    d["wqkv"] = nc.dram_tensor("wqkv", [C, 288], F32R, kind="ExternalInput")
    d["wo"] = nc.dram_tensor("wo", [C, C], F32R, kind="ExternalInput")
    d["wr"] = nc.dram_tensor("wr", [C, E], F32, kind="ExternalInput")
    d["br"] = nc.dram_tensor("br", [1, E], F32, kind="ExternalInput")
    d["w1"] = nc.dram_tensor("w1", [C, F1], BF16, kind="ExternalInput")
    d["w2"] = nc.dram_tensor("w2", [F1, C], BF16, kind="ExternalInput")
    d["ln1"] = nc.dram_tensor("ln1", [128, 6], F32, kind="ExternalInput")
    d["ln2"] = nc.dram_tensor("ln2", [128, 6], F32, kind="ExternalInput")
    d["ln2gb"] = nc.dram_tensor("ln2gb", [2, C], F32, kind="ExternalInput")
    d["cosq"] = nc.dram_tensor("cosq", [T, 96], F32, kind="ExternalInput")
    d["sinq"] = nc.dram_tensor("sinq", [T, 96], F32, kind="ExternalInput")
    d["cosk"] = nc.dram_tensor("cosk", [T, 24], F32, kind="ExternalInput")
    d["sink"] = nc.dram_tensor("sink", [T, 24], F32, kind="ExternalInput")
    d["wbase"] = nc.dram_tensor("wbase", [8, 1], F32, kind="ExternalInput")
    d["cb"] = nc.dram_tensor("cb", [128, 1], I32, kind="ExternalInput")
    d["orow"] = nc.dram_tensor("orow", [128, 3], I32, kind="ExternalInput")
    d["vmask"] = nc.dram_tensor("vmask", [1, 66], F32R, kind="ExternalInput")
    d["ones48"] = nc.dram_tensor("ones48", [1, 48], F32R, kind="ExternalInput")
    d["out"] = nc.dram_tensor("out", [TH, C], F32, kind="ExternalOutput")
    d["aux"] = nc.dram_tensor("aux", [1, 1], F32, kind="ExternalOutput")

    taps = {}

    def tapf(name, shape, dtype=F32):
        if not debug:
            return None
        taps[name] = nc.dram_tensor("tap_" + name, shape, dtype, kind="ExternalOutput")
        return taps[name]

    d["t_x2"] = tapf("x2", [TH, C])
    d["t_logits"] = tapf("logits", [TH, E])
    d["t_idx"] = tapf("idx", [TH, 1])
    d["t_pos"] = tapf("pos", [TH, 1])
    d["t_gate"] = tapf("gate", [TH, 1])
    d["t_flat"] = tapf("flat", [TH, C])
    d["t_inv"] = tapf("inv", [1, HALF])
    d["t_o2"] = tapf("o2", [HALF, C], BF16)
    d["debug"] = debug

    with tile.TileContext(nc) as tc:
        _body(nc, tc, d)
    nc.compile()
    return nc, taps


def _body(nc, tc, d):
    debug = d["debug"]
    with tc.tile_pool(name="persist", bufs=1) as persist, \
         tc.tile_pool(name="dram", bufs=1, space="DRAM") as dram:

        # ---------------- constants / weights ----------------
        ident = persist.tile([128, 128], F32, name="ident")
        make_identity(nc, ident[:])
        identb = persist.tile([128, 128], BF16, name="identb")
        make_identity(nc, identb[:])
        eps1 = persist.tile([128, 1], F32, name="eps1")
        nc.vector.memset(eps1[:], 1e-5)
        ones48 = persist.tile([1, 48], F32R, name="ones48")
        nc.sync.dma_start(out=ones48[:], in_=d["ones48"][:])
        ones8 = persist.tile([8, 1], F32, name="ones8")
        nc.vector.memset(ones8[:], 1.0)
        ones128r = persist.tile([1, 128], F32, name="ones128r")
        nc.vector.memset(ones128r[:], 1.0)
        ones128c = persist.tile([128, 1], F32, name="ones128c")
        nc.vector.memset(ones128c[:], 1.0)

        # strict upper-triangular ones: triu[j, i] = 1 iff j < i
        triu = persist.tile([128, 128], F32, name="triu")
        nc.gpsimd.memset(triu[:], 1.0)
        nc.gpsimd.affine_select(out=triu[:], in_=triu[:], pattern=[[1, 128]],
                                compare_op=OP.is_gt, fill=0.0, base=0,
                                channel_multiplier=-1)

        easc = persist.tile([128, 4], F32, name="easc")    # 0,1,2,3
        edesc = persist.tile([128, 4], F32, name="edesc")  # 4,3,2,1
        _ei = persist.tile([128, 4], I32, name="_ei")
        nc.gpsimd.iota(out=_ei[:], pattern=[[1, 4]], base=0, channel_multiplier=0)
        nc.vector.tensor_copy(out=easc[:], in_=_ei[:])
        _ei2 = persist.tile([128, 4], I32, name="_ei2")
        nc.gpsimd.iota(out=_ei2[:], pattern=[[-1, 4]], base=4, channel_multiplier=0)
        nc.vector.tensor_copy(out=edesc[:], in_=_ei2[:])

        wqkv_sb = [persist.tile([128, 288], F32R, name=f"wqkv{k}") for k in range(3)]
        wo_sb = [persist.tile([128, C], F32R, name=f"wo{k}") for k in range(3)]
        wr_sb = [persist.tile([128, E], F32, name=f"wr{k}") for k in range(3)]
        w1_sb = [persist.tile([128, F1], BF16, name=f"w1_{k}") for k in range(3)]
        w2_sb = [persist.tile([128, C], BF16, name=f"w2_{k}") for k in range(12)]
        for k in range(3):
            nc.sync.dma_start(out=wqkv_sb[k][:], in_=d["wqkv"][128 * k:128 * (k + 1), :])
            nc.sync.dma_start(out=wo_sb[k][:], in_=d["wo"][128 * k:128 * (k + 1), :])
            nc.sync.dma_start(out=wr_sb[k][:], in_=d["wr"][128 * k:128 * (k + 1), :])
            nc.sync.dma_start(out=w1_sb[k][:], in_=d["w1"][128 * k:128 * (k + 1), :])
        for k in range(12):
            nc.sync.dma_start(out=w2_sb[k][:], in_=d["w2"][128 * k:128 * (k + 1), :])
        br_sb = persist.tile([128, E], F32, name="br_sb")
        nc.sync.dma_start(out=br_sb[:],
                          in_=bass.AP(tensor=d["br"], offset=0, ap=[[0, 128], [1, E]]))
        ln1_sb = persist.tile([128, 6], F32, name="ln1_sb")
        nc.sync.dma_start(out=ln1_sb[:], in_=d["ln1"][:])
        ln2_sb = persist.tile([128, 6], F32, name="ln2_sb")
        nc.sync.dma_start(out=ln2_sb[:], in_=d["ln2"][:])
        g2bc = persist.tile([128, C], F32, name="g2bc")
        b2bc = persist.tile([128, C], F32, name="b2bc")
        nc.sync.dma_start(out=g2bc[:],
                          in_=bass.AP(tensor=d["ln2gb"], offset=0, ap=[[0, 128], [1, C]]))
        nc.sync.dma_start(out=b2bc[:],
                          in_=bass.AP(tensor=d["ln2gb"], offset=C, ap=[[0, 128], [1, C]]))
        wbase_sb = persist.tile([8, 1], F32, name="wbase_sb")
        nc.sync.dma_start(out=wbase_sb[:], in_=d["wbase"][:])
        cb_sb = persist.tile([128, 1], I32, name="cb_sb")
        nc.sync.dma_start(out=cb_sb[:], in_=d["cb"][:])

        # ====================== attention scope ======================
        with tc.tile_pool(name="abuf", bufs=1) as abuf:
            cosq_sb = abuf.tile([128, 96 * NTT], F32, name="cosq_sb")
            sinq_sb = abuf.tile([128, 96 * NTT], F32, name="sinq_sb")
            cosk_sb = abuf.tile([128, 24 * NTT], F32, name="cosk_sb")
            sink_sb = abuf.tile([128, 24 * NTT], F32, name="sink_sb")
            def _tab_ap(dt_, j):
                # sbuf[p, j*t + jj] = dram[128*t + p, jj]
                return bass.AP(tensor=dt_, offset=0,
                               ap=[[j, 128], [128 * j, NTT], [1, j]])
            nc.sync.dma_start(out=cosq_sb[:], in_=_tab_ap(d["cosq"], 96))
            nc.sync.dma_start(out=sinq_sb[:], in_=_tab_ap(d["sinq"], 96))
            nc.sync.dma_start(out=cosk_sb[:], in_=_tab_ap(d["cosk"], 24))
            nc.sync.dma_start(out=sink_sb[:], in_=_tab_ap(d["sink"], 24))

            # one wide causal mask; mask for diagonal position dd is the slice
            # [384-128*dd : 896-128*dd):  mask_wide[p, f'] = 1 iff f' - p >= 384
            mask_wide = abuf.tile([128, 896], F32R, name="mask_wide")
            with tc.tile_pool(name="mk", bufs=1) as mk:
                mf = mk.tile([128, 896], F32, name="mf")
                nc.gpsimd.memset(mf[:], 1.0)
                nc.gpsimd.affine_select(out=mf[:], in_=mf[:], pattern=[[1, 896]],
                                        compare_op=OP.is_ge, fill=0.0,
                                        base=-384, channel_multiplier=-1)
                nc.vector.tensor_copy(out=mask_wide[:], in_=mf[:])
            masks = [mask_wide[:, 384 - 128 * dd:896 - 128 * dd] for dd in range(4)]
            qT = [abuf.tile([48, T], F32R, name=f"qT{h}") for h in range(4)]
            kT = abuf.tile([48, T], F32R, name="kT")
            v_aug = abuf.tile([128, 66 * NTT], F32R, name="v_aug")
            nc.sync.dma_start(out=v_aug[:],
                              in_=bass.AP(tensor=d["vmask"], offset=0,
                                          ap=[[0, 128], [0, NTT], [1, 66]]))
            oT = [abuf.tile([48, T], F32R, name=f"oT{h}") for h in range(4)]

            # ---- P1-P3: LN1 -> hT -> QKV -> RoPE -> qT/kT/v_aug ----
            with tc.tile_pool(name="p1sb", bufs=3) as p1sb, \
                 tc.tile_pool(name="p1ps", bufs=2, space="PSUM") as p1ps, \
                 tc.tile_pool(name="p1ps2", bufs=2, space="PSUM") as p1ps2:
                for t in range(NTT):
                    xt = p1sb.tile([128, C], F32, name="xt")
                    nc.sync.dma_start(out=xt[:], in_=d["x"][128 * t:128 * (t + 1), :])
                    stats = p1sb.tile([128, 6], F32, name="stats")
                    nc.vector.bn_stats(out=stats[:], in_=xt[:])
                    mv = p1sb.tile([128, 2], F32, name="mv")
                    nc.vector.bn_aggr(out=mv[:], in_=stats[:])
                    rstd = p1sb.tile([128, 1], F32, name="rstd")
                    nc.scalar.activation(out=rstd[:], in_=mv[:, 1:2], func=AF.Sqrt,
                                         bias=eps1[:], scale=1.0)
                    nc.vector.reciprocal(out=rstd[:], in_=rstd[:])
                    xhn = p1sb.tile([128, C], F32, name="xhn")
                    nc.vector.tensor_scalar(out=xhn[:], in0=xt[:], scalar1=mv[:, 0:1],
                                            scalar2=rstd[:], op0=OP.subtract, op1=OP.mult)
                    hTt = []
                    for k in range(3):
                        ptr = p1ps.tile([128, 128], F32, name="ptr")
                        nc.tensor.transpose(out=ptr[:], in_=xhn[:, 128 * k:128 * (k + 1)],
                                            identity=ident[:])
                        hTk = p1sb.tile([128, 128], F32R, name=f"hTk{k}")
                        nc.vector.tensor_scalar(out=hTk[:], in0=ptr[:],
                                                scalar1=ln1_sb[:, k:k + 1],
                                                scalar2=ln1_sb[:, 3 + k:4 + k],
                                                op0=OP.mult, op1=OP.add)
                        hTt.append(hTk)
                    pq = p1ps2.tile([128, 288], F32, name="pq")
                    for k in range(3):
                        nc.tensor.matmul(out=pq[:],
                                         lhsT=hTt[k][:],
                                         rhs=wqkv_sb[k][:],
                                         start=(k == 0), stop=(k == 2))
                    qr = p1sb.tile([128, 192], F32, name="qr")
                    kr = p1sb.tile([128, 48], F32, name="kr")
                    sc1 = p1sb.tile([128, 96], F32, name="sc1")
                    sc2 = p1sb.tile([128, 96], F32, name="sc2")
                    cq = cosq_sb[:, 96 * t:96 * (t + 1)]
                    sq = sinq_sb[:, 96 * t:96 * (t + 1)]
                    ck = cosk_sb[:, 24 * t:24 * (t + 1)]
                    sk = sink_sb[:, 24 * t:24 * (t + 1)]
                    qe, qo = pq[:, 0:192:2], pq[:, 1:192:2]
                    nc.vector.tensor_tensor(out=sc1[:], in0=qe, in1=cq, op=OP.mult)
                    nc.vector.tensor_tensor(out=sc2[:], in0=qo, in1=sq, op=OP.mult)
                    nc.vector.tensor_tensor(out=qr[:, 0:192:2], in0=sc1[:], in1=sc2[:], op=OP.subtract)
                    nc.vector.tensor_tensor(out=sc1[:], in0=qe, in1=sq, op=OP.mult)
                    nc.vector.tensor_tensor(out=sc2[:], in0=qo, in1=cq, op=OP.mult)
                    nc.vector.tensor_tensor(out=qr[:, 1:192:2], in0=sc1[:], in1=sc2[:], op=OP.add)
                    ke, ko = pq[:, 192:240:2], pq[:, 193:240:2]
                    nc.vector.tensor_tensor(out=sc1[:, 0:24], in0=ke, in1=ck, op=OP.mult)
                    nc.vector.tensor_tensor(out=sc2[:, 0:24], in0=ko, in1=sk, op=OP.mult)
                    nc.vector.tensor_tensor(out=kr[:, 0:48:2], in0=sc1[:, 0:24], in1=sc2[:, 0:24], op=OP.subtract)
                    nc.vector.tensor_tensor(out=sc1[:, 0:24], in0=ke, in1=sk, op=OP.mult)
                    nc.vector.tensor_tensor(out=sc2[:, 0:24], in0=ko, in1=ck, op=OP.mult)
                    nc.vector.tensor_tensor(out=kr[:, 1:48:2], in0=sc1[:, 0:24], in1=sc2[:, 0:24], op=OP.add)
                    nc.scalar.activation(out=v_aug[:, 66 * t:66 * t + 48],
                                         in_=pq[:, 240:288], func=AF.Copy, scale=1.0)
                    for h in range(4):
                        ptq = p1ps.tile([48, 128], F32, name="ptq")
                        nc.tensor.transpose(out=ptq[:], in_=qr[:, 48 * h:48 * (h + 1)],
                                            identity=ident[:])
                        nc.vector.tensor_copy(out=qT[h][:, 128 * t:128 * (t + 1)], in_=ptq[:])
                    ptk = p1ps.tile([48, 128], F32, name="ptq")
                    nc.tensor.transpose(out=ptk[:], in_=kr[:], identity=ident[:])
                    nc.vector.tensor_copy(out=kT[:, 128 * t:128 * (t + 1)], in_=ptk[:])

            # ---- P4: attention (windows outer so oT halves finish early) ----
            agg_oT = dram.tile([768, TH], F32R)
            stage_a = dram.tile([192, TH], F32R)
            stage_b = dram.tile([192, TH], F32R)
            with tc.tile_pool(name="atsb", bufs=3) as atsb, \
                 tc.tile_pool(name="atps_s", bufs=2, space="PSUM") as atps_s, \
                 tc.tile_pool(name="atps_o", bufs=2, space="PSUM") as atps_o, \
                 tc.tile_pool(name="atps_b", bufs=2, space="PSUM") as atps_b:
                for w in range(4):
                    for h in range(4):
                        q0 = 512 * w
                        psum_o = atps_o.tile([66, 512], F32, name="psum_o")
                        ngrp = 2 * (w + 1)
                        pts = {}
                        for grp in range(ngrp + 1):
                            if grp < ngrp:
                                psum_s = atps_s.tile([128, 1024], F32, name="psum_s")
                                pt = atsb.tile([128, 1024], F32R, name="pt")
                                pts[grp] = pt
                                for i in range(2):
                                    j = 2 * grp + i
                                    nc.tensor.matmul(out=psum_s[:, 512 * i:512 * (i + 1)],
                                                     lhsT=kT[:, 128 * j:128 * (j + 1)],
                                                     rhs=qT[h][:, q0:q0 + 512],
                                                     start=True, stop=True)
                                nc.scalar.activation(out=pt[:], in_=psum_s[:], func=AF.Exp,
                                                     scale=INV_SQRT_HD)
                                for i in range(2):
                                    j = 2 * grp + i
                                    dd = j - 4 * w
                                    if dd >= 0:
                                        nc.vector.tensor_tensor(
                                            out=pt[:, 512 * i:512 * (i + 1)],
                                            in0=pt[:, 512 * i:512 * (i + 1)],
                                            in1=masks[dd], op=OP.mult)
                            if grp >= 1:
                                g2_ = grp - 1
                                ptp = pts.pop(g2_)
                                for i in range(2):
                                    j = 2 * g2_ + i
                                    nc.tensor.matmul(out=psum_o[:],
                                                     lhsT=v_aug[:, 66 * j:66 * j + 66],
                                                     rhs=ptp[:, 512 * i:512 * (i + 1)],
                                                     start=(g2_ == 0 and i == 0),
                                                     stop=(g2_ == ngrp - 1 and i == 1))
                        rec = atsb.tile([1, 512], F32R, name="rec")
                        with nc.allow_low_precision(reason="f32r softmax denom"):
                            nc.vector.reciprocal(out=rec[:], in_=psum_o[64:65, :])
                        psb = atps_b.tile([48, 512], F32, name="psb")
                        nc.tensor.matmul(out=psb[:], lhsT=ones48[:], rhs=rec[:],
                                         start=True, stop=True)
                        bc = atsb.tile([48, 512], F32, name="bc")
                        nc.vector.tensor_copy(out=bc[:], in_=psb[:])
                        nc.vector.tensor_tensor(out=oT[h][:, q0:q0 + 512],
                                                in0=psum_o[0:48, :], in1=bc[:], op=OP.mult)
                    # after windows 0-1 the first token-half of every head is
                    # done; AllGather it within the batch pair while windows
                    # 2-3 still compute.
                    if w == 1:
                        for h2 in range(4):
                            nc.sync.dma_start(out=stage_a[48 * h2:48 * (h2 + 1), :],
                                              in_=oT[h2][:, 0:TH])
                        nc.gpsimd.collective_compute(
                            "AllGather", OP.bypass,
                            replica_groups=[[0, 1], [2, 3], [4, 5], [6, 7]],
                            ins=[stage_a[:].opt()], outs=[agg_oT[0:384, :].opt()])
                    if w == 3:
                        for h2 in range(4):
                            nc.sync.dma_start(out=stage_b[48 * h2:48 * (h2 + 1), :],
                                              in_=oT[h2][:, TH:T])
                        nc.gpsimd.collective_compute(
                            "AllGather", OP.bypass,
                            replica_groups=[[0, 1], [2, 3], [4, 5], [6, 7]],
                            ins=[stage_b[:].opt()], outs=[agg_oT[384:768, :].opt()])
        # abuf closed

        # ---- P5: o-proj + residual -> x2; LN2 -> flat(+AG) and flatT ----
        # agg_oT rows [384*s + 48*(4*r + h) + dd] = head (4r+h) dim dd of
        # token-half s; my half is s == my pair rank, selected with the
        # per-core orow index vector (indirect gather).
        orow_sb = persist.tile([128, 3], I32, name="orow_sb")
        nc.sync.dma_start(out=orow_sb[:], in_=d["orow"][:])
        oTf = [persist.tile([128, TH], F32R, name=f"oTf{k}") for k in range(3)]
        for k in range(3):
            nc.gpsimd.indirect_dma_start(
                out=oTf[k][:], out_offset=None,
                in_=agg_oT[:],
                in_offset=bass.IndirectOffsetOnAxis(ap=orow_sb[:, k:k + 1], axis=0))
        xh_sb = [persist.tile([128, C], F32, name=f"xh{t}") for t in range(NHT)]
        for t in range(NHT):
            nc.sync.dma_start(out=xh_sb[t][:], in_=d["xh"][128 * t:128 * (t + 1), :])

        x2_sb = [persist.tile([128, C], F32, name=f"x2_{t}") for t in range(NHT)]
        flatT = [persist.tile([128, TH], F32, name=f"flatT{k}") for k in range(3)]
        flat_stage = dram.tile([TH, C], BF16)
        with tc.tile_pool(name="p5sb", bufs=3) as p5sb, \
             tc.tile_pool(name="p5ps", bufs=2, space="PSUM") as p5ps, \
             tc.tile_pool(name="p5ps2", bufs=2, space="PSUM") as p5ps2:
            for t in range(NHT):
                po = p5ps.tile([128, C], F32, name="po")
                for k in range(3):
                    nc.tensor.matmul(out=po[:],
                                     lhsT=oTf[k][:, 128 * t:128 * (t + 1)],
                                     rhs=wo_sb[k][:], start=(k == 0), stop=(k == 2))
                nc.vector.tensor_tensor(out=x2_sb[t][:], in0=po[:], in1=xh_sb[t][:], op=OP.add)
                if debug:
                    nc.sync.dma_start(out=d["t_x2"][128 * t:128 * (t + 1), :], in_=x2_sb[t][:])
                stats = p5sb.tile([128, 6], F32, name="stats")
                nc.vector.bn_stats(out=stats[:], in_=x2_sb[t][:])
                mv = p5sb.tile([128, 2], F32, name="mv")
                nc.vector.bn_aggr(out=mv[:], in_=stats[:])
                rstd = p5sb.tile([128, 1], F32, name="rstd")
                nc.scalar.activation(out=rstd[:], in_=mv[:, 1:2], func=AF.Sqrt,
                                     bias=eps1[:], scale=1.0)
                nc.vector.reciprocal(out=rstd[:], in_=rstd[:])
                xh2 = p5sb.tile([128, C], F32, name="xh2")
                nc.vector.tensor_scalar(out=xh2[:], in0=x2_sb[t][:], scalar1=mv[:, 0:1],
                                        scalar2=rstd[:], op0=OP.subtract, op1=OP.mult)
                fl = p5sb.tile([128, C], F32, name="fl")
                nc.vector.tensor_tensor(out=fl[:], in0=xh2[:], in1=g2bc[:], op=OP.mult)
                nc.vector.tensor_tensor(out=fl[:], in0=fl[:], in1=b2bc[:], op=OP.add)
                flb = p5sb.tile([128, C], BF16, name="flb")
                nc.vector.tensor_copy(out=flb[:], in_=fl[:])
                nc.sync.dma_start(out=flat_stage[128 * t:128 * (t + 1), :], in_=flb[:])
                if debug:
                    nc.sync.dma_start(out=d["t_flat"][128 * t:128 * (t + 1), :], in_=fl[:])
                for k in range(3):
                    ptr = p5ps2.tile([128, 128], F32, name="ptr")
                    nc.tensor.transpose(out=ptr[:], in_=xh2[:, 128 * k:128 * (k + 1)],
                                        identity=ident[:])
                    nc.vector.tensor_scalar(out=flatT[k][:, 128 * t:128 * (t + 1)],
                                            in0=ptr[:],
                                            scalar1=ln2_sb[:, k:k + 1],
                                            scalar2=ln2_sb[:, 3 + k:4 + k],
                                            op0=OP.mult, op1=OP.add)

        # flat AllGather is issued AFTER the (tiny) counts AllGather so the
        # counts result isn't queued behind 6 MB of flat traffic; the flat AG
        # then overlaps the position/scatter phase.  Empty slots carry index
        # ZROW (out of bounds): the gather skips them and the pre-zeroed
        # destination supplies the zero row.
        flat_full = dram.tile([ZROW, C], BF16, addr_space="Shared")

        # ---- P7: router; P8: counts AG + aux; P9: positions/slots/scatter;
        #      P10: inverse-map exchange ----
        gate_t = [persist.tile([128, 1], F32, name=f"gate{t}") for t in range(NHT)]
        sclip_t = [persist.tile([128, 1], I32, name=f"sclip{t}") for t in range(NHT)]
        invT = persist.tile([128, HALF // 128], I32, name="invT")
        with tc.tile_pool(name="rtsb", bufs=3) as rtsb, \
             tc.tile_pool(name="rtper", bufs=1) as rtper, \
             tc.tile_pool(name="rtps", bufs=1, space="PSUM") as rtps, \
             tc.tile_pool(name="rtpsc", bufs=1, space="PSUM") as rtpsc:
            idx_t, onehot_t, tv_t, cnt_t = [], [], [], []
            psum_c0 = rtpsc.tile([1, 4], F32, name="psum_c0")
            psum_c1 = rtpsc.tile([1, 4], F32, name="psum_c1")
            for t in range(NHT):
                pl = rtps.tile([128, E], F32, name="pl")
                for k in range(3):
                    nc.tensor.matmul(out=pl[:], lhsT=flatT[k][:, 128 * t:128 * (t + 1)],
                                     rhs=wr_sb[k][:], start=(k == 0), stop=(k == 2))
                lg = rtsb.tile([128, E], F32, name="lg")
                nc.vector.tensor_tensor(out=lg[:], in0=pl[:], in1=br_sb[:], op=OP.add)
                if debug:
                    nc.sync.dma_start(out=d["t_logits"][128 * t:128 * (t + 1), :], in_=lg[:])
                m = rtsb.tile([128, 1], F32, name="m")
                nc.vector.reduce_max(out=m[:], in_=lg[:], axis=AX.X)
                negm = rtsb.tile([128, 1], F32, name="negm")
                nc.vector.tensor_scalar(out=negm[:], in0=m[:], scalar1=-1.0,
                                        scalar2=None, op0=OP.mult)
                pu = rtsb.tile([128, E], F32, name="pu")
                z = rtsb.tile([128, 1], F32, name="z")
                nc.scalar.activation(out=pu[:], in_=lg[:], func=AF.Exp, bias=negm[:],
                                     scale=1.0, accum_out=z[:])
                tv = rtper.tile([128, 1], F32, name=f"tv{t}")
                nc.vector.reciprocal(out=tv[:], in_=z[:])
                probs = rtsb.tile([128, E], F32, name="probs")
                nc.vector.tensor_scalar(out=probs[:], in0=pu[:], scalar1=tv[:],
                                        scalar2=None, op0=OP.mult)
                eq = rtsb.tile([128, E], F32, name="eq")
                nc.vector.tensor_scalar(out=eq[:], in0=lg[:], scalar1=m[:],
                                        scalar2=None, op0=OP.is_ge)
                wt = rtsb.tile([128, E], F32, name="wt")
                nc.vector.tensor_tensor(out=wt[:], in0=eq[:], in1=edesc[:], op=OP.mult)
                rmax = rtsb.tile([128, 1], F32, name="rmax")
                nc.vector.reduce_max(out=rmax[:], in_=wt[:], axis=AX.X)
                idx = rtper.tile([128, 1], F32, name=f"idx{t}")
                nc.vector.tensor_scalar(out=idx[:], in0=rmax[:], scalar1=-1.0,
                                        scalar2=4.0, op0=OP.mult, op1=OP.add)
                oh = rtper.tile([128, E], F32, name=f"oh{t}")
                nc.vector.tensor_tensor(out=oh[:], in0=idx[:].to_broadcast([128, E]),
                                        in1=easc[:], op=OP.is_equal)
                # per-tile expert counts (PE colsum) + running global sums
                pcnt = rtps.tile([1, E], F32, name="pcnt")
                nc.tensor.matmul(out=pcnt[:], lhsT=ones128c[:], rhs=oh[:],
                                 start=True, stop=True)
                cnt = rtper.tile([1, E], F32, name=f"cnt{t}")
                nc.vector.tensor_copy(out=cnt[:], in_=pcnt[:])
                nc.tensor.matmul(out=psum_c0[:], lhsT=ones128c[:], rhs=oh[:],
                                 start=(t == 0), stop=(t == NHT - 1))
                nc.tensor.matmul(out=psum_c1[:], lhsT=ones128c[:], rhs=probs[:],
                                 start=(t == 0), stop=(t == NHT - 1))
                idx_t.append(idx); onehot_t.append(oh); tv_t.append(tv); cnt_t.append(cnt)
                if debug:
                    nc.sync.dma_start(out=d["t_idx"][128 * t:128 * (t + 1), :], in_=idx[:])

            counts_loc = rtper.tile([1, 8], F32, name="counts_loc")
            nc.vector.tensor_copy(out=counts_loc[:, 0:4], in_=psum_c0[:])
            nc.vector.tensor_copy(out=counts_loc[:, 4:8], in_=psum_c1[:])

            # AG#1 counts + prob sums
            ag1_in = dram.tile([1, 8], F32)
            ag1_out = dram.tile([8, 8], F32, addr_space="Shared")
            nc.sync.dma_start(out=ag1_in[:], in_=counts_loc[:])
            nc.gpsimd.collective_compute(
                "AllGather", OP.bypass, replica_groups=[list(range(N_CORES))],
                ins=[ag1_in[:].opt()], outs=[ag1_out[:].opt()])
            nc.gpsimd.collective_compute(
                "AllGather", OP.bypass, replica_groups=[list(range(N_CORES))],
                ins=[flat_stage[:].opt()], outs=[flat_full[:].opt()])
            ag_sb = rtper.tile([8, 8], F32, name="ag_sb")
            nc.sync.dma_start(out=ag_sb[:], in_=ag1_out[:])
            pbase = rtps.tile([1, E], F32, name="pbase")
            nc.tensor.matmul(out=pbase[:], lhsT=wbase_sb[:], rhs=ag_sb[:, 0:4],
                             start=True, stop=True)
            base_sb = rtper.tile([1, E], F32, name="base_sb")
            nc.vector.tensor_copy(out=base_sb[:], in_=pbase[:])
            psums = rtps.tile([1, 8], F32, name="psums")
            nc.tensor.matmul(out=psums[:], lhsT=ones8[:], rhs=ag_sb[:], start=True, stop=True)
            aux_sb = rtper.tile([1, 1], F32, name="aux_sb")
            cmin = rtper.tile([1, E], F32, name="cmin")
            nc.vector.tensor_scalar(out=cmin[:], in0=psums[:, 0:4], scalar1=float(CAP),
                                    scalar2=None, op0=OP.min)
            smul = rtper.tile([1, E], F32, name="smul")
            nc.vector.tensor_tensor(out=smul[:], in0=cmin[:], in1=psums[:, 4:8], op=OP.mult)
            nc.vector.reduce_sum(out=aux_sb[:], in_=smul[:], axis=AX.X)
            nc.vector.tensor_scalar(out=aux_sb[:], in0=aux_sb[:],
                                    scalar1=float(E) / float(ZROW) ** 2,
                                    scalar2=None, op0=OP.mult)
            nc.sync.dma_start(out=d["aux"][:], in_=aux_sb[:])

            # ---- P9: global positions ----
            inv_local = dram.tile([INV_ROWS, 1], I32)
            zi = rtper.tile([128, INV_ROWS // 128], I32, name="zi")
            nc.vector.memset(zi[:], 0)
            nc.sync.dma_start(
                out=bass.AP(tensor=inv_local.tensor, offset=0,
                            ap=[[INV_ROWS // 128, 128], [1, INV_ROWS // 128]]),
                in_=zi[:])
            r_run = base_sb
            for t in range(NHT):
                ppos = rtps.tile([128, E], F32, name="ppos")
                nc.tensor.matmul(out=ppos[:], lhsT=triu[:], rhs=onehot_t[t][:],
                                 start=True, stop=False)
                nc.tensor.matmul(out=ppos[:], lhsT=ones128r[:], rhs=r_run[:],
                                 start=False, stop=True)
                nr = rtper.tile([1, E], F32, name=f"nr{t}")
                nc.vector.tensor_tensor(out=nr[:], in0=r_run[:], in1=cnt_t[t][:], op=OP.add)
                r_run = nr
                sc = rtsb.tile([128, E], F32, name="sc")
                nc.vector.tensor_tensor(out=sc[:], in0=ppos[:], in1=onehot_t[t][:], op=OP.mult)
                pos = rtsb.tile([128, 1], F32, name="pos")
                nc.vector.reduce_sum(out=pos[:], in_=sc[:], axis=AX.X)
                if debug:
                    nc.sync.dma_start(out=d["t_pos"][128 * t:128 * (t + 1), :], in_=pos[:])
                sbase = rtsb.tile([128, 1], F32, name="sbase")
                nc.vector.tensor_scalar(out=sbase[:], in0=idx_t[t][:], scalar1=float(CAP),
                                        scalar2=None, op0=OP.mult)
                dd_ = rtsb.tile([128, 1], F32, name="dd_")
                nc.vector.tensor_scalar(out=dd_[:], in0=pos[:], scalar1=float(CAP),
                                        scalar2=None, op0=OP.is_lt)
                nc.vector.tensor_tensor(out=gate_t[t][:], in0=tv_t[t][:], in1=dd_[:], op=OP.mult)
                if debug:
                    nc.sync.dma_start(out=d["t_gate"][128 * t:128 * (t + 1), :], in_=gate_t[t][:])
                slot = rtsb.tile([128, 1], F32, name="slot")
                nc.vector.tensor_tensor(out=slot[:], in0=sbase[:], in1=pos[:], op=OP.add)
                se = rtsb.tile([128, 1], F32, name="se")
                nc.vector.tensor_scalar(out=se[:], in0=slot[:], scalar1=float(DUMPED),
                                        scalar2=None, op0=OP.subtract)
                nc.vector.tensor_tensor(out=se[:], in0=se[:], in1=dd_[:], op=OP.mult)
                nc.vector.tensor_scalar(out=se[:], in0=se[:], scalar1=float(DUMPED),
                                        scalar2=None, op0=OP.add)
                sei = rtsb.tile([128, 1], I32, name="sei")
                nc.vector.tensor_copy(out=sei[:], in_=se[:])
                pclip = rtsb.tile([128, 1], F32, name="pclip")
                nc.vector.tensor_scalar(out=pclip[:], in0=pos[:], scalar1=float(CAP - 1),
                                        scalar2=None, op0=OP.min)
                scf = rtsb.tile([128, 1], F32, name="scf")
                nc.vector.tensor_tensor(out=scf[:], in0=sbase[:], in1=pclip[:], op=OP.add)
                nc.vector.tensor_copy(out=sclip_t[t][:], in_=scf[:])
                ti = rtsb.tile([128, 1], I32, name="ti")
                nc.gpsimd.iota(out=ti[:], pattern=[[0, 1]], base=128 * t + 1,
                               channel_multiplier=1)
                tid = rtsb.tile([128, 1], I32, name="tid")
                nc.vector.tensor_tensor(out=tid[:], in0=ti[:], in1=cb_sb[:], op=OP.add)
                nc.gpsimd.indirect_dma_start(
                    out=bass.AP(tensor=inv_local.tensor, offset=0,
                                ap=[[1, E * CAP], [1, 1]]),
                    out_offset=bass.IndirectOffsetOnAxis(ap=sei[:, 0:1], axis=0),
                    in_=tid[:, 0:1], in_offset=None,
                    bounds_check=E * CAP - 1, oob_is_err=False)

            # ---- P10: inverse-map exchange, select my slot range ----
            a2a2_out = dram.tile([8, HALF], I32)
            nc.gpsimd.collective_compute(
                "AllToAll", OP.bypass, replica_groups=[list(range(N_CORES))],
                ins=[bass.AP(tensor=inv_local.tensor, offset=0,
                             ap=[[HALF, 8], [1, HALF]]).opt()],
                outs=[a2a2_out[:].opt()])
            c8i = rtper.tile([8, HALF], I32, name="c8i")
            c8f = rtper.tile([8, HALF], F32, name="c8f")
            nc.sync.dma_start(out=c8i[:], in_=a2a2_out[:])
            nc.vector.tensor_copy(out=c8f[:], in_=c8i[:])
            myinv_f = rtper.tile([1, HALF], F32, name="myinv_f")
            for n in range(0, HALF, 512):
                nn = min(512, HALF - n)
                pinv = rtps.tile([1, 512], F32, name="pinv")
                nc.tensor.matmul(out=pinv[:, 0:nn], lhsT=ones8[:], rhs=c8f[:, n:n + nn],
                                 start=True, stop=True)
                nc.vector.tensor_scalar(out=myinv_f[:, n:n + nn], in0=pinv[:, 0:nn],
                                        scalar1=1.0, scalar2=None, op0=OP.subtract)
            msk = rtper.tile([1, HALF], F32, name="msk")
            nc.vector.tensor_scalar(out=msk[:], in0=myinv_f[:], scalar1=0.0,
                                    scalar2=float(ZROW + 1), op0=OP.is_lt, op1=OP.mult)
            nc.vector.tensor_tensor(out=myinv_f[:], in0=myinv_f[:], in1=msk[:], op=OP.add)
            if debug:
                nc.sync.dma_start(out=d["t_inv"][:], in_=myinv_f[:])
            myinv_i = rtper.tile([1, HALF], I32, name="myinv_i")
            nc.vector.tensor_copy(out=myinv_i[:], in_=myinv_f[:])
            # SBUF partitions are physical: bounce through DRAM to re-partition
            inv_dram = dram.tile([1, HALF], I32)
            nc.sync.dma_start(out=inv_dram[:], in_=myinv_i[:])
            nc.sync.dma_start(out=invT[:],
                              in_=bass.AP(tensor=inv_dram.tensor, offset=0,
                                          ap=[[1, 128], [128, HALF // 128]]))

        # ---- P11: expert FFN over my 1280 slots ----
        o2_stage = dram.tile([HALF, C], BF16)
        with tc.tile_pool(name="mosb", bufs=3) as mosb, \
             tc.tile_pool(name="moper", bufs=1) as moper, \
             tc.tile_pool(name="mops", bufs=2, space="PSUM") as mops, \
             tc.tile_pool(name="mops2", bufs=2, space="PSUM") as mops2:
            bufT = [moper.tile([128, HALF], BF16, name=f"bufT{k}") for k in range(3)]
            for kt in range(HALF // 128):
                gb = mosb.tile([128, C], BF16, name="gb")
                nc.vector.memset(gb[:], 0.0)
                nc.gpsimd.indirect_dma_start(
                    out=gb[:], out_offset=None,
                    in_=flat_full[:],
                    in_offset=bass.IndirectOffsetOnAxis(ap=invT[:, kt:kt + 1], axis=0),
                    bounds_check=ZROW - 1, oob_is_err=False)
                for k in range(3):
                    ptr = mops2.tile([128, 128], BF16, name="ptr")
                    nc.tensor.transpose(out=ptr[:], in_=gb[:, 128 * k:128 * (k + 1)],
                                        identity=identb[:])
                    nc.vector.tensor_copy(out=bufT[k][:, 128 * kt:128 * (kt + 1)], in_=ptr[:])
            h1T = [moper.tile([128, HALF], BF16, name=f"h1T{m}") for m in range(12)]
            nwin = [(0, 512), (512, 1024), (1024, 1280)]
            for m in range(12):
                ph = mops.tile([128, HALF], F32, name="ph")
                for (n0, n1) in nwin:
                    for k in range(3):
                        nc.tensor.matmul(out=ph[:, n0:n1],
                                         lhsT=w1_sb[k][:, 128 * m:128 * (m + 1)],
                                         rhs=bufT[k][:, n0:n1],
                                         start=(k == 0), stop=(k == 2))
                nc.scalar.activation(out=h1T[m][:], in_=ph[:], func=AF.Relu, scale=1.0)
            o2T = [moper.tile([128, HALF], BF16, name=f"o2T{m}") for m in range(3)]
            for m in range(3):
                ph = mops.tile([128, HALF], F32, name="ph")
                for (n0, n1) in nwin:
                    for k in range(12):
                        nc.tensor.matmul(out=ph[:, n0:n1],
                                         lhsT=w2_sb[k][:, 128 * m:128 * (m + 1)],
                                         rhs=h1T[k][:, n0:n1],
                                         start=(k == 0), stop=(k == 11))
                nc.vector.tensor_copy(out=o2T[m][:], in_=ph[:])
            for kt in range(HALF // 128):
                ob = mosb.tile([128, C], BF16, name="ob")
                for m in range(3):
                    ptr = mops2.tile([128, 128], BF16, name="ptr")
                    nc.tensor.transpose(out=ptr[:], in_=o2T[m][:, 128 * kt:128 * (kt + 1)],
                                        identity=identb[:])
                    nc.vector.tensor_copy(out=ob[:, 128 * m:128 * (m + 1)], in_=ptr[:])
                nc.sync.dma_start(out=o2_stage[128 * kt:128 * (kt + 1), :], in_=ob[:])
                if debug:
                    nc.sync.dma_start(out=d["t_o2"][128 * kt:128 * (kt + 1), :], in_=ob[:])

        o2_full = dram.tile([E * CAP, C], BF16, addr_space="Shared")
        nc.gpsimd.collective_compute(
            "AllGather", OP.bypass, replica_groups=[list(range(N_CORES))],
            ins=[o2_stage[:].opt()], outs=[o2_full[:].opt()])

        # ---- P12: final gather + gate + residual ----
        with tc.tile_pool(name="fsb", bufs=3) as fsb:
            for t in range(NHT):
                og = fsb.tile([128, C], BF16, name="og")
                nc.gpsimd.indirect_dma_start(
                    out=og[:], out_offset=None,
                    in_=o2_full[:],
                    in_offset=bass.IndirectOffsetOnAxis(ap=sclip_t[t][:, 0:1], axis=0))
                sg = fsb.tile([128, C], F32, name="sg")
                nc.vector.tensor_scalar(out=sg[:], in0=og[:], scalar1=gate_t[t][:],
                                        scalar2=None, op0=OP.mult)
                ot = fsb.tile([128, C], F32, name="ot")
                nc.vector.tensor_tensor(out=ot[:], in0=sg[:], in1=x2_sb[t][:], op=OP.add)
                nc.sync.dma_start(out=d["out"][128 * t:128 * (t + 1), :], in_=ot[:])


# ---------------------------------------------------------------------------
# Host side
# ---------------------------------------------------------------------------

def _fold(W, A, B_):
    return (np.asarray(W, np.float64)
            + SCALE * (np.asarray(A, np.float64).T @ np.asarray(B_, np.float64).T)
            ).astype(np.float32)


def _rope_tables():
    inv = 1.0 / (10000.0 ** (np.arange(0, HD, 2, dtype=np.float64) / HD))
    ang = np.arange(T, dtype=np.float64)[:, None] * inv
    cos = np.cos(ang).astype(np.float32)
    sin = np.sin(ang).astype(np.float32)
    return np.tile(cos, (1, 4)), np.tile(sin, (1, 4)), cos, sin


def _vmask():
    v = np.zeros((1, 66), np.float32)
    v[0, 64] = 1.0
    return v


def make_in_maps(inputs):
    x = np.asarray(inputs["x"], np.float32)
    aid = int(np.asarray(inputs["adapter_id"]))
    Wq = _fold(inputs["Wq"], np.asarray(inputs["Aq"])[aid], np.asarray(inputs["Bq"])[aid])
    Wk = _fold(inputs["Wk"], np.asarray(inputs["Ak"])[aid], np.asarray(inputs["Bk"])[aid])
    Wv = _fold(inputs["Wv"], np.asarray(inputs["Av"])[aid], np.asarray(inputs["Bv"])[aid])
    Wo = _fold(inputs["Wo"], np.asarray(inputs["Ao"])[aid], np.asarray(inputs["Bo"])[aid])
    Wr = np.ascontiguousarray(np.asarray(inputs["Wr"], np.float32))
    br = np.asarray(inputs["br"], np.float32).reshape(1, E)
    W1 = [_fold(np.asarray(inputs["W1"])[e], np.asarray(inputs["A1"])[e, aid],
                np.asarray(inputs["B1"])[e, aid]) for e in range(E)]
    W2 = [_fold(np.asarray(inputs["W2"])[e], np.asarray(inputs["A2"])[e, aid],
                np.asarray(inputs["B2"])[e, aid]) for e in range(E)]
    ln1g = np.asarray(inputs["ln1_g"], np.float32)
    ln1b = np.asarray(inputs["ln1_b"], np.float32)
    ln2g = np.asarray(inputs["ln2_g"], np.float32)
    ln2b = np.asarray(inputs["ln2_b"], np.float32)
    ln1 = np.concatenate([ln1g.reshape(3, 128).T, ln1b.reshape(3, 128).T], 1)
    ln2 = np.concatenate([ln2g.reshape(3, 128).T, ln2b.reshape(3, 128).T], 1)
    ln2gb = np.stack([ln2g, ln2b])
    cosq, sinq, cosk, sink = _rope_tables()

    in_maps = []
    for c in range(N_CORES):
        b, g = c // 2, c % 2
        e = c // 2
        wqkv = np.concatenate([
            Wq[:, 192 * g:192 * (g + 1)],
            Wk[:, 48 * g:48 * (g + 1)],
            Wv[:, 48 * g:48 * (g + 1)],
        ], axis=1)
        in_maps.append({
            "x": np.ascontiguousarray(x[b]),
            "xh": np.ascontiguousarray(x[b, TH * g:TH * (g + 1)]),
            "wqkv": np.ascontiguousarray(wqkv),
            "wo": Wo,
            "wr": Wr,
            "br": br,
            "w1": W1[e].astype(ml_dtypes.bfloat16),
            "w2": W2[e].astype(ml_dtypes.bfloat16),
            "ln1": np.ascontiguousarray(ln1),
            "ln2": np.ascontiguousarray(ln2),
            "ln2gb": np.ascontiguousarray(ln2gb),
            "cosq": np.ascontiguousarray(cosq),
            "sinq": np.ascontiguousarray(sinq),
            "cosk": np.ascontiguousarray(cosk),
            "sink": np.ascontiguousarray(sink),
            "wbase": (np.arange(8) < c).astype(np.float32).reshape(8, 1),
            "vmask": _vmask(),
            "ones48": np.ones((1, 48), np.float32),
            "cb": np.full((128, 1), TH * c, np.int32),
            "orow": (384 * g + 128 * np.arange(3)[None, :]
                     + np.arange(128)[:, None]).astype(np.int32),
        })
    return in_maps


_CACHED = {}


def _get_nc(debug=False):
    key = bool(debug)
    if key not in _CACHED:
        _CACHED[key] = build(key)
    return _CACHED[key]


def assemble(results):
    full = np.concatenate([results[c]["out"] for c in range(N_CORES)], 0)
    out = full.reshape(B, T, C)
    aux = np.float32(results[0]["aux"][0, 0])
    return out, aux


def kernel(**inputs):
    nc, _ = _get_nc(False)
    in_maps = make_in_maps(inputs)
    res = bass_utils.run_bass_kernel_spmd(nc, in_maps, core_ids=list(range(N_CORES)))
    return assemble(res.results)


# revision 21
# speedup vs baseline: 1.0284x; 1.0213x over previous
"""Trainium2 Bass kernel for nn_BlockLoRA (GQA attention + LoRA + capacity-routed
top-1 MoE), SPMD over 8 NeuronCores.

Sharding: core c = 2*b + g computes batch b's attention for q-heads
[4g, 4g+4) and kv-head g.  Attention-output head-halves are exchanged
pairwise (AllToAll), after which core c owns global tokens
[1024*c, 1024*(c+1)).  The MoE phase is expert-parallel: core c runs
expert c//2 on capacity slots [1280*c, 1280*(c+1)); dispatch uses an
AllGather of the LN2 output plus an AllToAll'd slot->token inverse map
built with indirect-DMA scatters.
"""

import sys

for _p in ("/opt/trn_rl_repo", "/root/.axon_site/_ro/trn_rl_repo"):
    if _p not in sys.path:
        sys.path.insert(0, _p)

import math

import numpy as np
import ml_dtypes

import concourse.bass as bass
import concourse.bacc as bacc
import concourse.tile as tile
from concourse import mybir
from concourse import bass_utils
from concourse.masks import make_identity

F32 = mybir.dt.float32
F32R = mybir.dt.float32r
BF16 = mybir.dt.bfloat16
I32 = mybir.dt.int32
AX = mybir.AxisListType
OP = mybir.AluOpType
AF = mybir.ActivationFunctionType

C = 384
HQ = 8
HKV = 2
HD = C // HQ          # 48
R = 4
E = 4
T = 2048
B = 4
N_CORES = 8
TH = T // 2           # 1024 tokens per core in phase B
CAP = int(math.ceil(1.25 * B * T / E))   # 2560
HALF = CAP // 2       # 1280 slots per core
F1 = 4 * C            # 1536
SCALE = 1.0 / R
INV_SQRT_HD = 1.0 / math.sqrt(HD)
NTT = T // 128        # 16 token tiles over the full batch
NHT = TH // 128       # 8 token tiles over my half
DUMPED = 999999       # scatter index for dropped tokens
ZROW = N_CORES * TH   # 8192: index of the all-zero row in flat_full
INV_ROWS = (E * CAP // 128 + 1) * 128    # 10368

DEBUG = False


def build(debug=DEBUG):
    nc = bacc.Bacc("TRN2", target_bir_lowering=False, debug=False,
                   num_devices=N_CORES)

    d = {}
    d["x"] = nc.dram_tensor("x", [T, C], F32, kind="ExternalInput")
    d["xh"] = nc.dram_tensor("xh", [TH, C], F32, kind="ExternalInput")
    d["wqkv"] = nc.dram_tensor("wqkv", [C, 288], F32R, kind="ExternalInput")
    d["wo"] = nc.dram_tensor("wo", [C, C], F32R, kind="ExternalInput")
    d["wr"] = nc.dram_tensor("wr", [C, E], F32, kind="ExternalInput")
    d["br"] = nc.dram_tensor("br", [1, E], F32, kind="ExternalInput")
    d["w1"] = nc.dram_tensor("w1", [C, F1], BF16, kind="ExternalInput")
    d["w2"] = nc.dram_tensor("w2", [F1, C], BF16, kind="ExternalInput")
    d["ln1"] = nc.dram_tensor("ln1", [128, 6], F32, kind="ExternalInput")
    d["ln2"] = nc.dram_tensor("ln2", [128, 6], F32, kind="ExternalInput")
    d["ln2gb"] = nc.dram_tensor("ln2gb", [2, C], F32, kind="ExternalInput")
    d["cosq"] = nc.dram_tensor("cosq", [T, 96], F32, kind="ExternalInput")
    d["sinq"] = nc.dram_tensor("sinq", [T, 96], F32, kind="ExternalInput")
    d["cosk"] = nc.dram_tensor("cosk", [T, 24], F32, kind="ExternalInput")
    d["sink"] = nc.dram_tensor("sink", [T, 24], F32, kind="ExternalInput")
    d["wbase"] = nc.dram_tensor("wbase", [8, 1], F32, kind="ExternalInput")
    d["cb"] = nc.dram_tensor("cb", [128, 1], I32, kind="ExternalInput")
    d["orow"] = nc.dram_tensor("orow", [128, 3], I32, kind="ExternalInput")
    d["vmask"] = nc.dram_tensor("vmask", [1, 66], F32R, kind="ExternalInput")
    d["ones48"] = nc.dram_tensor("ones48", [1, 48], F32R, kind="ExternalInput")
    d["out"] = nc.dram_tensor("out", [TH, C], F32, kind="ExternalOutput")
    d["aux"] = nc.dram_tensor("aux", [1, 1], F32, kind="ExternalOutput")

    taps = {}

    def tapf(name, shape, dtype=F32):
        if not debug:
            return None
        taps[name] = nc.dram_tensor("tap_" + name, shape, dtype, kind="ExternalOutput")
        return taps[name]

    d["t_x2"] = tapf("x2", [TH, C])
    d["t_logits"] = tapf("logits", [TH, E])
    d["t_idx"] = tapf("idx", [TH, 1])
    d["t_pos"] = tapf("pos", [TH, 1])
    d["t_gate"] = tapf("gate", [TH, 1])
    d["t_flat"] = tapf("flat", [TH, C])
    d["t_inv"] = tapf("inv", [1, HALF])
    d["t_o2"] = tapf("o2", [HALF, C], BF16)
    d["debug"] = debug

    with tile.TileContext(nc) as tc:
        _body(nc, tc, d)
    nc.compile()
    return nc, taps


def _body(nc, tc, d):
    debug = d["debug"]
    with tc.tile_pool(name="persist", bufs=1) as persist, \
         tc.tile_pool(name="dram", bufs=1, space="DRAM") as dram:

        # ---------------- constants / weights ----------------
        ident = persist.tile([128, 128], F32, name="ident")
        make_identity(nc, ident[:])
        identb = persist.tile([128, 128], BF16, name="identb")
        make_identity(nc, identb[:])
        eps1 = persist.tile([128, 1], F32, name="eps1")
        nc.vector.memset(eps1[:], 1e-5)
        ones48 = persist.tile([1, 48], F32R, name="ones48")
        nc.sync.dma_start(out=ones48[:], in_=d["ones48"][:])
        ones8 = persist.tile([8, 1], F32, name="ones8")
        nc.vector.memset(ones8[:], 1.0)
        ones128r = persist.tile([1, 128], F32, name="ones128r")
        nc.vector.memset(ones128r[:], 1.0)
        ones128c = persist.tile([128, 1], F32, name="ones128c")
        nc.vector.memset(ones128c[:], 1.0)

        # strict upper-triangular ones: triu[j, i] = 1 iff j < i
        triu = persist.tile([128, 128], F32, name="triu")
        nc.gpsimd.memset(triu[:], 1.0)
        nc.gpsimd.affine_select(out=triu[:], in_=triu[:], pattern=[[1, 128]],
                                compare_op=OP.is_gt, fill=0.0, base=0,
                                channel_multiplier=-1)

        easc = persist.tile([128, 4], F32, name="easc")    # 0,1,2,3
        edesc = persist.tile([128, 4], F32, name="edesc")  # 4,3,2,1
        _ei = persist.tile([128, 4], I32, name="_ei")
        nc.gpsimd.iota(out=_ei[:], pattern=[[1, 4]], base=0, channel_multiplier=0)
        nc.vector.tensor_copy(out=easc[:], in_=_ei[:])
        _ei2 = persist.tile([128, 4], I32, name="_ei2")
        nc.gpsimd.iota(out=_ei2[:], pattern=[[-1, 4]], base=4, channel_multiplier=0)
        nc.vector.tensor_copy(out=edesc[:], in_=_ei2[:])

        wqkv_sb = [persist.tile([128, 288], F32R, name=f"wqkv{k}") for k in range(3)]
        wo_sb = [persist.tile([128, C], F32R, name=f"wo{k}") for k in range(3)]
        wr_sb = [persist.tile([128, E], F32, name=f"wr{k}") for k in range(3)]
        w1_sb = [persist.tile([128, F1], BF16, name=f"w1_{k}") for k in range(3)]
        w2_sb = [persist.tile([128, C], BF16, name=f"w2_{k}") for k in range(12)]
        for k in range(3):
            nc.sync.dma_start(out=wqkv_sb[k][:], in_=d["wqkv"][128 * k:128 * (k + 1), :])
            nc.sync.dma_start(out=wo_sb[k][:], in_=d["wo"][128 * k:128 * (k + 1), :])
            nc.sync.dma_start(out=wr_sb[k][:], in_=d["wr"][128 * k:128 * (k + 1), :])
            nc.sync.dma_start(out=w1_sb[k][:], in_=d["w1"][128 * k:128 * (k + 1), :])
        for k in range(12):
            nc.sync.dma_start(out=w2_sb[k][:], in_=d["w2"][128 * k:128 * (k + 1), :])
        br_sb = persist.tile([128, E], F32, name="br_sb")
        nc.sync.dma_start(out=br_sb[:],
                          in_=bass.AP(tensor=d["br"], offset=0, ap=[[0, 128], [1, E]]))
        ln1_sb = persist.tile([128, 6], F32, name="ln1_sb")
        nc.sync.dma_start(out=ln1_sb[:], in_=d["ln1"][:])
        ln2_sb = persist.tile([128, 6], F32, name="ln2_sb")
        nc.sync.dma_start(out=ln2_sb[:], in_=d["ln2"][:])
        g2bc = persist.tile([128, C], F32, name="g2bc")
        b2bc = persist.tile([128, C], F32, name="b2bc")
        nc.sync.dma_start(out=g2bc[:],
                          in_=bass.AP(tensor=d["ln2gb"], offset=0, ap=[[0, 128], [1, C]]))
        nc.sync.dma_start(out=b2bc[:],
                          in_=bass.AP(tensor=d["ln2gb"], offset=C, ap=[[0, 128], [1, C]]))
        wbase_sb = persist.tile([8, 1], F32, name="wbase_sb")
        nc.sync.dma_start(out=wbase_sb[:], in_=d["wbase"][:])
        cb_sb = persist.tile([128, 1], I32, name="cb_sb")
        nc.sync.dma_start(out=cb_sb[:], in_=d["cb"][:])

        # ====================== attention scope ======================
        with tc.tile_pool(name="abuf", bufs=1) as abuf:
            cosq_sb = abuf.tile([128, 96 * NTT], F32, name="cosq_sb")
            sinq_sb = abuf.tile([128, 96 * NTT], F32, name="sinq_sb")
            cosk_sb = abuf.tile([128, 24 * NTT], F32, name="cosk_sb")
            sink_sb = abuf.tile([128, 24 * NTT], F32, name="sink_sb")
            def _tab_ap(dt_, j):
                # sbuf[p, j*t + jj] = dram[128*t + p, jj]
                return bass.AP(tensor=dt_, offset=0,
                               ap=[[j, 128], [128 * j, NTT], [1, j]])
            nc.sync.dma_start(out=cosq_sb[:], in_=_tab_ap(d["cosq"], 96))
            nc.sync.dma_start(out=sinq_sb[:], in_=_tab_ap(d["sinq"], 96))
            nc.sync.dma_start(out=cosk_sb[:], in_=_tab_ap(d["cosk"], 24))
            nc.sync.dma_start(out=sink_sb[:], in_=_tab_ap(d["sink"], 24))

            # one wide causal mask; mask for diagonal position dd is the slice
            # [384-128*dd : 896-128*dd):  mask_wide[p, f'] = 1 iff f' - p >= 384
            mask_wide = abuf.tile([128, 896], F32R, name="mask_wide")
            with tc.tile_pool(name="mk", bufs=1) as mk:
                mf = mk.tile([128, 896], F32, name="mf")
                nc.gpsimd.memset(mf[:], 1.0)
                nc.gpsimd.affine_select(out=mf[:], in_=mf[:], pattern=[[1, 896]],
                                        compare_op=OP.is_ge, fill=0.0,
                                        base=-384, channel_multiplier=-1)
                nc.vector.tensor_copy(out=mask_wide[:], in_=mf[:])
            masks = [mask_wide[:, 384 - 128 * dd:896 - 128 * dd] for dd in range(4)]
            qT = [abuf.tile([48, T], F32R, name=f"qT{h}") for h in range(4)]
            kT = abuf.tile([48, T], F32R, name="kT")
            v_aug = abuf.tile([128, 66 * NTT], F32R, name="v_aug")
            nc.sync.dma_start(out=v_aug[:],
                              in_=bass.AP(tensor=d["vmask"], offset=0,
                                          ap=[[0, 128], [0, NTT], [1, 66]]))
            oT = [abuf.tile([48, T], F32R, name=f"oT{h}") for h in range(4)]

            # ---- P1-P3: LN1 -> hT -> QKV -> RoPE -> qT/kT/v_aug ----
            with tc.tile_pool(name="p1sb", bufs=3) as p1sb, \
                 tc.tile_pool(name="p1ps", bufs=2, space="PSUM") as p1ps, \
                 tc.tile_pool(name="p1ps2", bufs=2, space="PSUM") as p1ps2:
                for t in range(NTT):
                    xt = p1sb.tile([128, C], F32, name="xt")
                    nc.sync.dma_start(out=xt[:], in_=d["x"][128 * t:128 * (t + 1), :])
                    stats = p1sb.tile([128, 6], F32, name="stats")
                    nc.vector.bn_stats(out=stats[:], in_=xt[:])
                    mv = p1sb.tile([128, 2], F32, name="mv")
                    nc.vector.bn_aggr(out=mv[:], in_=stats[:])
                    rstd = p1sb.tile([128, 1], F32, name="rstd")
                    nc.scalar.activation(out=rstd[:], in_=mv[:, 1:2], func=AF.Sqrt,
                                         bias=eps1[:], scale=1.0)
                    nc.vector.reciprocal(out=rstd[:], in_=rstd[:])
                    xhn = p1sb.tile([128, C], F32, name="xhn")
                    nc.vector.tensor_scalar(out=xhn[:], in0=xt[:], scalar1=mv[:, 0:1],
                                            scalar2=rstd[:], op0=OP.subtract, op1=OP.mult)
                    hTt = []
                    for k in range(3):
                        ptr = p1ps.tile([128, 128], F32, name="ptr")
                        nc.tensor.transpose(out=ptr[:], in_=xhn[:, 128 * k:128 * (k + 1)],
                                            identity=ident[:])
                        hTk = p1sb.tile([128, 128], F32R, name=f"hTk{k}")
                        nc.vector.tensor_scalar(out=hTk[:], in0=ptr[:],
                                                scalar1=ln1_sb[:, k:k + 1],
                                                scalar2=ln1_sb[:, 3 + k:4 + k],
                                                op0=OP.mult, op1=OP.add)
                        hTt.append(hTk)
                    pq = p1ps2.tile([128, 288], F32, name="pq")
                    for k in range(3):
                        nc.tensor.matmul(out=pq[:],
                                         lhsT=hTt[k][:],
                                         rhs=wqkv_sb[k][:],
                                         start=(k == 0), stop=(k == 2))
                    qr = p1sb.tile([128, 192], F32, name="qr")
                    kr = p1sb.tile([128, 48], F32, name="kr")
                    sc1 = p1sb.tile([128, 96], F32, name="sc1")
                    sc2 = p1sb.tile([128, 96], F32, name="sc2")
                    cq = cosq_sb[:, 96 * t:96 * (t + 1)]
                    sq = sinq_sb[:, 96 * t:96 * (t + 1)]
                    ck = cosk_sb[:, 24 * t:24 * (t + 1)]
                    sk = sink_sb[:, 24 * t:24 * (t + 1)]
                    qe, qo = pq[:, 0:192:2], pq[:, 1:192:2]
                    nc.vector.tensor_tensor(out=sc1[:], in0=qe, in1=cq, op=OP.mult)
                    nc.vector.tensor_tensor(out=sc2[:], in0=qo, in1=sq, op=OP.mult)
                    nc.vector.tensor_tensor(out=qr[:, 0:192:2], in0=sc1[:], in1=sc2[:], op=OP.subtract)
                    nc.vector.tensor_tensor(out=sc1[:], in0=qe, in1=sq, op=OP.mult)
                    nc.vector.tensor_tensor(out=sc2[:], in0=qo, in1=cq, op=OP.mult)
                    nc.vector.tensor_tensor(out=qr[:, 1:192:2], in0=sc1[:], in1=sc2[:], op=OP.add)
                    ke, ko = pq[:, 192:240:2], pq[:, 193:240:2]
                    nc.vector.tensor_tensor(out=sc1[:, 0:24], in0=ke, in1=ck, op=OP.mult)
                    nc.vector.tensor_tensor(out=sc2[:, 0:24], in0=ko, in1=sk, op=OP.mult)
                    nc.vector.tensor_tensor(out=kr[:, 0:48:2], in0=sc1[:, 0:24], in1=sc2[:, 0:24], op=OP.subtract)
                    nc.vector.tensor_tensor(out=sc1[:, 0:24], in0=ke, in1=sk, op=OP.mult)
                    nc.vector.tensor_tensor(out=sc2[:, 0:24], in0=ko, in1=ck, op=OP.mult)
                    nc.vector.tensor_tensor(out=kr[:, 1:48:2], in0=sc1[:, 0:24], in1=sc2[:, 0:24], op=OP.add)
                    nc.scalar.activation(out=v_aug[:, 66 * t:66 * t + 48],
                                         in_=pq[:, 240:288], func=AF.Copy, scale=1.0)
                    for h in range(4):
                        ptq = p1ps.tile([48, 128], F32, name="ptq")
                        nc.tensor.transpose(out=ptq[:], in_=qr[:, 48 * h:48 * (h + 1)],
                                            identity=ident[:])
                        nc.vector.tensor_copy(out=qT[h][:, 128 * t:128 * (t + 1)], in_=ptq[:])
                    ptk = p1ps.tile([48, 128], F32, name="ptq")
                    nc.tensor.transpose(out=ptk[:], in_=kr[:], identity=ident[:])
                    nc.vector.tensor_copy(out=kT[:, 128 * t:128 * (t + 1)], in_=ptk[:])

            # ---- P4: attention (windows outer so oT halves finish early) ----
            agg_oT = dram.tile([768, TH], F32R)
            stage_a = dram.tile([192, TH], F32R)
            stage_b = dram.tile([192, TH], F32R)
            with tc.tile_pool(name="atsb", bufs=3) as atsb, \
                 tc.tile_pool(name="atps_s", bufs=2, space="PSUM") as atps_s, \
                 tc.tile_pool(name="atps_o", bufs=2, space="PSUM") as atps_o, \
                 tc.tile_pool(name="atps_b", bufs=2, space="PSUM") as atps_b:
                for w in range(4):
                    for h in range(4):
                        q0 = 512 * w
                        psum_o = atps_o.tile([66, 512], F32, name="psum_o")
                        ngrp = 2 * (w + 1)
                        pts = {}
                        for grp in range(ngrp + 1):
                            if grp < ngrp:
                                psum_s = atps_s.tile([128, 1024], F32, name="psum_s")
                                pt = atsb.tile([128, 1024], F32R, name="pt", bufs=5)
                                pts[grp] = pt
                                for i in range(2):
                                    j = 2 * grp + i
                                    nc.tensor.matmul(out=psum_s[:, 512 * i:512 * (i + 1)],
                                                     lhsT=kT[:, 128 * j:128 * (j + 1)],
                                                     rhs=qT[h][:, q0:q0 + 512],
                                                     start=True, stop=True)
                                nc.scalar.activation(out=pt[:], in_=psum_s[:], func=AF.Exp,
                                                     scale=INV_SQRT_HD)
                                for i in range(2):
                                    j = 2 * grp + i
                                    dd = j - 4 * w
                                    if dd >= 0:
                                        nc.vector.tensor_tensor(
                                            out=pt[:, 512 * i:512 * (i + 1)],
                                            in0=pt[:, 512 * i:512 * (i + 1)],
                                            in1=masks[dd], op=OP.mult)
                            if grp >= 1:
                                g2_ = grp - 1
                                ptp = pts.pop(g2_)
                                for i in range(2):
                                    j = 2 * g2_ + i
                                    nc.tensor.matmul(out=psum_o[:],
                                                     lhsT=v_aug[:, 66 * j:66 * j + 66],
                                                     rhs=ptp[:, 512 * i:512 * (i + 1)],
                                                     start=(g2_ == 0 and i == 0),
                                                     stop=(g2_ == ngrp - 1 and i == 1))
                        rec = atsb.tile([1, 512], F32R, name="rec")
                        with nc.allow_low_precision(reason="f32r softmax denom"):
                            nc.vector.reciprocal(out=rec[:], in_=psum_o[64:65, :])
                        psb = atps_b.tile([48, 512], F32, name="psb")
                        nc.tensor.matmul(out=psb[:], lhsT=ones48[:], rhs=rec[:],
                                         start=True, stop=True)
                        bc = atsb.tile([48, 512], F32, name="bc")
                        nc.vector.tensor_copy(out=bc[:], in_=psb[:])
                        nc.vector.tensor_tensor(out=oT[h][:, q0:q0 + 512],
                                                in0=psum_o[0:48, :], in1=bc[:], op=OP.mult)
                    # after windows 0-1 the first token-half of every head is
                    # done; AllGather it within the batch pair while windows
                    # 2-3 still compute.
                    if w == 1:
                        for h2 in range(4):
                            nc.sync.dma_start(out=stage_a[48 * h2:48 * (h2 + 1), :],
                                              in_=oT[h2][:, 0:TH])
                        nc.gpsimd.collective_compute(
                            "AllGather", OP.bypass,
                            replica_groups=[[0, 1], [2, 3], [4, 5], [6, 7]],
                            ins=[stage_a[:].opt()], outs=[agg_oT[0:384, :].opt()])
                    if w == 3:
                        for h2 in range(4):
                            nc.sync.dma_start(out=stage_b[48 * h2:48 * (h2 + 1), :],
                                              in_=oT[h2][:, TH:T])
                        nc.gpsimd.collective_compute(
                            "AllGather", OP.bypass,
                            replica_groups=[[0, 1], [2, 3], [4, 5], [6, 7]],
                            ins=[stage_b[:].opt()], outs=[agg_oT[384:768, :].opt()])
        # abuf closed

        # ---- P5: o-proj + residual -> x2; LN2 -> flat(+AG) and flatT ----
        # agg_oT rows [384*s + 48*(4*r + h) + dd] = head (4r+h) dim dd of
        # token-half s; my half is s == my pair rank, selected with the
        # per-core orow index vector (indirect gather).
        orow_sb = persist.tile([128, 3], I32, name="orow_sb")
        nc.sync.dma_start(out=orow_sb[:], in_=d["orow"][:])
        oTf = [persist.tile([128, TH], F32R, name=f"oTf{k}") for k in range(3)]
        for k in range(3):
            nc.gpsimd.indirect_dma_start(
                out=oTf[k][:], out_offset=None,
                in_=agg_oT[:],
                in_offset=bass.IndirectOffsetOnAxis(ap=orow_sb[:, k:k + 1], axis=0))
        xh_sb = [persist.tile([128, C], F32, name=f"xh{t}") for t in range(NHT)]
        for t in range(NHT):
            nc.sync.dma_start(out=xh_sb[t][:], in_=d["xh"][128 * t:128 * (t + 1), :])

        x2_sb = [persist.tile([128, C], F32, name=f"x2_{t}") for t in range(NHT)]
        flatT = [persist.tile([128, TH], F32, name=f"flatT{k}") for k in range(3)]
        flat_stage = dram.tile([TH, C], BF16)
        with tc.tile_pool(name="p5sb", bufs=3) as p5sb, \
             tc.tile_pool(name="p5ps", bufs=2, space="PSUM") as p5ps, \
             tc.tile_pool(name="p5ps2", bufs=2, space="PSUM") as p5ps2:
            for t in range(NHT):
                po = p5ps.tile([128, C], F32, name="po")
                for k in range(3):
                    nc.tensor.matmul(out=po[:],
                                     lhsT=oTf[k][:, 128 * t:128 * (t + 1)],
                                     rhs=wo_sb[k][:], start=(k == 0), stop=(k == 2))
                nc.vector.tensor_tensor(out=x2_sb[t][:], in0=po[:], in1=xh_sb[t][:], op=OP.add)
                if debug:
                    nc.sync.dma_start(out=d["t_x2"][128 * t:128 * (t + 1), :], in_=x2_sb[t][:])
                stats = p5sb.tile([128, 6], F32, name="stats")
                nc.vector.bn_stats(out=stats[:], in_=x2_sb[t][:])
                mv = p5sb.tile([128, 2], F32, name="mv")
                nc.vector.bn_aggr(out=mv[:], in_=stats[:])
                rstd = p5sb.tile([128, 1], F32, name="rstd")
                nc.scalar.activation(out=rstd[:], in_=mv[:, 1:2], func=AF.Sqrt,
                                     bias=eps1[:], scale=1.0)
                nc.vector.reciprocal(out=rstd[:], in_=rstd[:])
                xh2 = p5sb.tile([128, C], F32, name="xh2")
                nc.vector.tensor_scalar(out=xh2[:], in0=x2_sb[t][:], scalar1=mv[:, 0:1],
                                        scalar2=rstd[:], op0=OP.subtract, op1=OP.mult)
                fl = p5sb.tile([128, C], F32, name="fl")
                nc.vector.tensor_tensor(out=fl[:], in0=xh2[:], in1=g2bc[:], op=OP.mult)
                nc.vector.tensor_tensor(out=fl[:], in0=fl[:], in1=b2bc[:], op=OP.add)
                flb = p5sb.tile([128, C], BF16, name="flb")
                nc.vector.tensor_copy(out=flb[:], in_=fl[:])
                nc.sync.dma_start(out=flat_stage[128 * t:128 * (t + 1), :], in_=flb[:])
                if debug:
                    nc.sync.dma_start(out=d["t_flat"][128 * t:128 * (t + 1), :], in_=fl[:])
                for k in range(3):
                    ptr = p5ps2.tile([128, 128], F32, name="ptr")
                    nc.tensor.transpose(out=ptr[:], in_=xh2[:, 128 * k:128 * (k + 1)],
                                        identity=ident[:])
                    nc.vector.tensor_scalar(out=flatT[k][:, 128 * t:128 * (t + 1)],
                                            in0=ptr[:],
                                            scalar1=ln2_sb[:, k:k + 1],
                                            scalar2=ln2_sb[:, 3 + k:4 + k],
                                            op0=OP.mult, op1=OP.add)

        # flat AllGather is issued AFTER the (tiny) counts AllGather so the
        # counts result isn't queued behind 6 MB of flat traffic; the flat AG
        # then overlaps the position/scatter phase.  Empty slots carry index
        # ZROW (out of bounds): the gather skips them and the pre-zeroed
        # destination supplies the zero row.
        flat_full = dram.tile([ZROW, C], BF16, addr_space="Shared")

        # ---- P7: router; P8: counts AG + aux; P9: positions/slots/scatter;
        #      P10: inverse-map exchange ----
        gate_t = [persist.tile([128, 1], F32, name=f"gate{t}") for t in range(NHT)]
        sclip_t = [persist.tile([128, 1], I32, name=f"sclip{t}") for t in range(NHT)]
        invT = persist.tile([128, HALF // 128], I32, name="invT")
        with tc.tile_pool(name="rtsb", bufs=3) as rtsb, \
             tc.tile_pool(name="rtper", bufs=1) as rtper, \
             tc.tile_pool(name="rtps", bufs=1, space="PSUM") as rtps, \
             tc.tile_pool(name="rtpsc", bufs=1, space="PSUM") as rtpsc:
            idx_t, onehot_t, tv_t, cnt_t = [], [], [], []
            psum_c0 = rtpsc.tile([1, 4], F32, name="psum_c0")
            psum_c1 = rtpsc.tile([1, 4], F32, name="psum_c1")
            for t in range(NHT):
                pl = rtps.tile([128, E], F32, name="pl")
                for k in range(3):
                    nc.tensor.matmul(out=pl[:], lhsT=flatT[k][:, 128 * t:128 * (t + 1)],
                                     rhs=wr_sb[k][:], start=(k == 0), stop=(k == 2))
                lg = rtsb.tile([128, E], F32, name="lg")
                nc.vector.tensor_tensor(out=lg[:], in0=pl[:], in1=br_sb[:], op=OP.add)
                if debug:
                    nc.sync.dma_start(out=d["t_logits"][128 * t:128 * (t + 1), :], in_=lg[:])
                m = rtsb.tile([128, 1], F32, name="m")
                nc.vector.reduce_max(out=m[:], in_=lg[:], axis=AX.X)
                negm = rtsb.tile([128, 1], F32, name="negm")
                nc.vector.tensor_scalar(out=negm[:], in0=m[:], scalar1=-1.0,
                                        scalar2=None, op0=OP.mult)
                pu = rtsb.tile([128, E], F32, name="pu")
                z = rtsb.tile([128, 1], F32, name="z")
                nc.scalar.activation(out=pu[:], in_=lg[:], func=AF.Exp, bias=negm[:],
                                     scale=1.0, accum_out=z[:])
                tv = rtper.tile([128, 1], F32, name=f"tv{t}")
                nc.vector.reciprocal(out=tv[:], in_=z[:])
                probs = rtsb.tile([128, E], F32, name="probs")
                nc.vector.tensor_scalar(out=probs[:], in0=pu[:], scalar1=tv[:],
                                        scalar2=None, op0=OP.mult)
                eq = rtsb.tile([128, E], F32, name="eq")
                nc.vector.tensor_scalar(out=eq[:], in0=lg[:], scalar1=m[:],
                                        scalar2=None, op0=OP.is_ge)
                wt = rtsb.tile([128, E], F32, name="wt")
                nc.vector.tensor_tensor(out=wt[:], in0=eq[:], in1=edesc[:], op=OP.mult)
                rmax = rtsb.tile([128, 1], F32, name="rmax")
                nc.vector.reduce_max(out=rmax[:], in_=wt[:], axis=AX.X)
                idx = rtper.tile([128, 1], F32, name=f"idx{t}")
                nc.vector.tensor_scalar(out=idx[:], in0=rmax[:], scalar1=-1.0,
                                        scalar2=4.0, op0=OP.mult, op1=OP.add)
                oh = rtper.tile([128, E], F32, name=f"oh{t}")
                nc.vector.tensor_tensor(out=oh[:], in0=idx[:].to_broadcast([128, E]),
                                        in1=easc[:], op=OP.is_equal)
                # per-tile expert counts (PE colsum) + running global sums
                pcnt = rtps.tile([1, E], F32, name="pcnt")
                nc.tensor.matmul(out=pcnt[:], lhsT=ones128c[:], rhs=oh[:],
                                 start=True, stop=True)
                cnt = rtper.tile([1, E], F32, name=f"cnt{t}")
                nc.vector.tensor_copy(out=cnt[:], in_=pcnt[:])
                nc.tensor.matmul(out=psum_c0[:], lhsT=ones128c[:], rhs=oh[:],
                                 start=(t == 0), stop=(t == NHT - 1))
                nc.tensor.matmul(out=psum_c1[:], lhsT=ones128c[:], rhs=probs[:],
                                 start=(t == 0), stop=(t == NHT - 1))
                idx_t.append(idx); onehot_t.append(oh); tv_t.append(tv); cnt_t.append(cnt)
                if debug:
                    nc.sync.dma_start(out=d["t_idx"][128 * t:128 * (t + 1), :], in_=idx[:])

            counts_loc = rtper.tile([1, 8], F32, name="counts_loc")
            nc.vector.tensor_copy(out=counts_loc[:, 0:4], in_=psum_c0[:])
            nc.vector.tensor_copy(out=counts_loc[:, 4:8], in_=psum_c1[:])

            # AG#1 counts + prob sums
            ag1_in = dram.tile([1, 8], F32)
            ag1_out = dram.tile([8, 8], F32, addr_space="Shared")
            nc.sync.dma_start(out=ag1_in[:], in_=counts_loc[:])
            nc.gpsimd.collective_compute(
                "AllGather", OP.bypass, replica_groups=[list(range(N_CORES))],
                ins=[ag1_in[:].opt()], outs=[ag1_out[:].opt()])
            nc.gpsimd.collective_compute(
                "AllGather", OP.bypass, replica_groups=[list(range(N_CORES))],
                ins=[flat_stage[:].opt()], outs=[flat_full[:].opt()])
            ag_sb = rtper.tile([8, 8], F32, name="ag_sb")
            nc.sync.dma_start(out=ag_sb[:], in_=ag1_out[:])
            pbase = rtps.tile([1, E], F32, name="pbase")
            nc.tensor.matmul(out=pbase[:], lhsT=wbase_sb[:], rhs=ag_sb[:, 0:4],
                             start=True, stop=True)
            base_sb = rtper.tile([1, E], F32, name="base_sb")
            nc.vector.tensor_copy(out=base_sb[:], in_=pbase[:])
            psums = rtps.tile([1, 8], F32, name="psums")
            nc.tensor.matmul(out=psums[:], lhsT=ones8[:], rhs=ag_sb[:], start=True, stop=True)
            aux_sb = rtper.tile([1, 1], F32, name="aux_sb")
            cmin = rtper.tile([1, E], F32, name="cmin")
            nc.vector.tensor_scalar(out=cmin[:], in0=psums[:, 0:4], scalar1=float(CAP),
                                    scalar2=None, op0=OP.min)
            smul = rtper.tile([1, E], F32, name="smul")
            nc.vector.tensor_tensor(out=smul[:], in0=cmin[:], in1=psums[:, 4:8], op=OP.mult)
            nc.vector.reduce_sum(out=aux_sb[:], in_=smul[:], axis=AX.X)
            nc.vector.tensor_scalar(out=aux_sb[:], in0=aux_sb[:],
                                    scalar1=float(E) / float(ZROW) ** 2,
                                    scalar2=None, op0=OP.mult)
            nc.sync.dma_start(out=d["aux"][:], in_=aux_sb[:])

            # ---- P9: global positions ----
            inv_local = dram.tile([INV_ROWS, 1], I32)
            zi = rtper.tile([128, INV_ROWS // 128], I32, name="zi")
            nc.vector.memset(zi[:], 0)
            nc.sync.dma_start(
                out=bass.AP(tensor=inv_local.tensor, offset=0,
                            ap=[[INV_ROWS // 128, 128], [1, INV_ROWS // 128]]),
                in_=zi[:])
            r_run = base_sb
            for t in range(NHT):
                ppos = rtps.tile([128, E], F32, name="ppos")
                nc.tensor.matmul(out=ppos[:], lhsT=triu[:], rhs=onehot_t[t][:],
                                 start=True, stop=False)
                nc.tensor.matmul(out=ppos[:], lhsT=ones128r[:], rhs=r_run[:],
                                 start=False, stop=True)
                nr = rtper.tile([1, E], F32, name=f"nr{t}")
                nc.vector.tensor_tensor(out=nr[:], in0=r_run[:], in1=cnt_t[t][:], op=OP.add)
                r_run = nr
                sc = rtsb.tile([128, E], F32, name="sc")
                nc.vector.tensor_tensor(out=sc[:], in0=ppos[:], in1=onehot_t[t][:], op=OP.mult)
                pos = rtsb.tile([128, 1], F32, name="pos")
                nc.vector.reduce_sum(out=pos[:], in_=sc[:], axis=AX.X)
                if debug:
                    nc.sync.dma_start(out=d["t_pos"][128 * t:128 * (t + 1), :], in_=pos[:])
                sbase = rtsb.tile([128, 1], F32, name="sbase")
                nc.vector.tensor_scalar(out=sbase[:], in0=idx_t[t][:], scalar1=float(CAP),
                                        scalar2=None, op0=OP.mult)
                dd_ = rtsb.tile([128, 1], F32, name="dd_")
                nc.vector.tensor_scalar(out=dd_[:], in0=pos[:], scalar1=float(CAP),
                                        scalar2=None, op0=OP.is_lt)
                nc.vector.tensor_tensor(out=gate_t[t][:], in0=tv_t[t][:], in1=dd_[:], op=OP.mult)
                if debug:
                    nc.sync.dma_start(out=d["t_gate"][128 * t:128 * (t + 1), :], in_=gate_t[t][:])
                slot = rtsb.tile([128, 1], F32, name="slot")
                nc.vector.tensor_tensor(out=slot[:], in0=sbase[:], in1=pos[:], op=OP.add)
                se = rtsb.tile([128, 1], F32, name="se")
                nc.vector.tensor_scalar(out=se[:], in0=slot[:], scalar1=float(DUMPED),
                                        scalar2=None, op0=OP.subtract)
                nc.vector.tensor_tensor(out=se[:], in0=se[:], in1=dd_[:], op=OP.mult)
                nc.vector.tensor_scalar(out=se[:], in0=se[:], scalar1=float(DUMPED),
                                        scalar2=None, op0=OP.add)
                sei = rtsb.tile([128, 1], I32, name="sei")
                nc.vector.tensor_copy(out=sei[:], in_=se[:])
                pclip = rtsb.tile([128, 1], F32, name="pclip")
                nc.vector.tensor_scalar(out=pclip[:], in0=pos[:], scalar1=float(CAP - 1),
                                        scalar2=None, op0=OP.min)
                scf = rtsb.tile([128, 1], F32, name="scf")
                nc.vector.tensor_tensor(out=scf[:], in0=sbase[:], in1=pclip[:], op=OP.add)
                nc.vector.tensor_copy(out=sclip_t[t][:], in_=scf[:])
                ti = rtsb.tile([128, 1], I32, name="ti")
                nc.gpsimd.iota(out=ti[:], pattern=[[0, 1]], base=128 * t + 1,
                               channel_multiplier=1)
                tid = rtsb.tile([128, 1], I32, name="tid")
                nc.vector.tensor_tensor(out=tid[:], in0=ti[:], in1=cb_sb[:], op=OP.add)
                nc.gpsimd.indirect_dma_start(
                    out=bass.AP(tensor=inv_local.tensor, offset=0,
                                ap=[[1, E * CAP], [1, 1]]),
                    out_offset=bass.IndirectOffsetOnAxis(ap=sei[:, 0:1], axis=0),
                    in_=tid[:, 0:1], in_offset=None,
                    bounds_check=E * CAP - 1, oob_is_err=False)

            # ---- P10: inverse-map exchange, select my slot range ----
            a2a2_out = dram.tile([8, HALF], I32)
            nc.gpsimd.collective_compute(
                "AllToAll", OP.bypass, replica_groups=[list(range(N_CORES))],
                ins=[bass.AP(tensor=inv_local.tensor, offset=0,
                             ap=[[HALF, 8], [1, HALF]]).opt()],
                outs=[a2a2_out[:].opt()])
            c8i = rtper.tile([8, HALF], I32, name="c8i")
            c8f = rtper.tile([8, HALF], F32, name="c8f")
            nc.sync.dma_start(out=c8i[:], in_=a2a2_out[:])
            nc.vector.tensor_copy(out=c8f[:], in_=c8i[:])
            myinv_f = rtper.tile([1, HALF], F32, name="myinv_f")
            for n in range(0, HALF, 512):
                nn = min(512, HALF - n)
                pinv = rtps.tile([1, 512], F32, name="pinv")
                nc.tensor.matmul(out=pinv[:, 0:nn], lhsT=ones8[:], rhs=c8f[:, n:n + nn],
                                 start=True, stop=True)
                nc.vector.tensor_scalar(out=myinv_f[:, n:n + nn], in0=pinv[:, 0:nn],
                                        scalar1=1.0, scalar2=None, op0=OP.subtract)
            msk = rtper.tile([1, HALF], F32, name="msk")
            nc.vector.tensor_scalar(out=msk[:], in0=myinv_f[:], scalar1=0.0,
                                    scalar2=float(ZROW + 1), op0=OP.is_lt, op1=OP.mult)
            nc.vector.tensor_tensor(out=myinv_f[:], in0=myinv_f[:], in1=msk[:], op=OP.add)
            if debug:
                nc.sync.dma_start(out=d["t_inv"][:], in_=myinv_f[:])
            myinv_i = rtper.tile([1, HALF], I32, name="myinv_i")
            nc.vector.tensor_copy(out=myinv_i[:], in_=myinv_f[:])
            # SBUF partitions are physical: bounce through DRAM to re-partition
            inv_dram = dram.tile([1, HALF], I32)
            nc.sync.dma_start(out=inv_dram[:], in_=myinv_i[:])
            nc.sync.dma_start(out=invT[:],
                              in_=bass.AP(tensor=inv_dram.tensor, offset=0,
                                          ap=[[1, 128], [128, HALF // 128]]))

        # ---- P11: expert FFN over my 1280 slots ----
        o2_stage = dram.tile([HALF, C], BF16)
        with tc.tile_pool(name="mosb", bufs=3) as mosb, \
             tc.tile_pool(name="moper", bufs=1) as moper, \
             tc.tile_pool(name="mops", bufs=2, space="PSUM") as mops, \
             tc.tile_pool(name="mops2", bufs=2, space="PSUM") as mops2:
            bufT = [moper.tile([128, HALF], BF16, name=f"bufT{k}") for k in range(3)]
            for kt in range(HALF // 128):
                gb = mosb.tile([128, C], BF16, name="gb")
                nc.vector.memset(gb[:], 0.0)
                nc.gpsimd.indirect_dma_start(
                    out=gb[:], out_offset=None,
                    in_=flat_full[:],
                    in_offset=bass.IndirectOffsetOnAxis(ap=invT[:, kt:kt + 1], axis=0),
                    bounds_check=ZROW - 1, oob_is_err=False)
                for k in range(3):
                    ptr = mops2.tile([128, 128], BF16, name="ptr")
                    nc.tensor.transpose(out=ptr[:], in_=gb[:, 128 * k:128 * (k + 1)],
                                        identity=identb[:])
                    nc.vector.tensor_copy(out=bufT[k][:, 128 * kt:128 * (kt + 1)], in_=ptr[:])
            h1T = [moper.tile([128, HALF], BF16, name=f"h1T{m}") for m in range(12)]
            nwin = [(0, 512), (512, 1024), (1024, 1280)]
            for m in range(12):
                ph = mops.tile([128, HALF], F32, name="ph")
                for (n0, n1) in nwin:
                    for k in range(3):
                        nc.tensor.matmul(out=ph[:, n0:n1],
                                         lhsT=w1_sb[k][:, 128 * m:128 * (m + 1)],
                                         rhs=bufT[k][:, n0:n1],
                                         start=(k == 0), stop=(k == 2))
                nc.scalar.activation(out=h1T[m][:], in_=ph[:], func=AF.Relu, scale=1.0)
            o2T = [moper.tile([128, HALF], BF16, name=f"o2T{m}") for m in range(3)]
            for m in range(3):
                ph = mops.tile([128, HALF], F32, name="ph")
                for (n0, n1) in nwin:
                    for k in range(12):
                        nc.tensor.matmul(out=ph[:, n0:n1],
                                         lhsT=w2_sb[k][:, 128 * m:128 * (m + 1)],
                                         rhs=h1T[k][:, n0:n1],
                                         start=(k == 0), stop=(k == 11))
                nc.vector.tensor_copy(out=o2T[m][:], in_=ph[:])
            for kt in range(HALF // 128):
                ob = mosb.tile([128, C], BF16, name="ob")
                for m in range(3):
                    ptr = mops2.tile([128, 128], BF16, name="ptr")
                    nc.tensor.transpose(out=ptr[:], in_=o2T[m][:, 128 * kt:128 * (kt + 1)],
                                        identity=identb[:])
                    nc.vector.tensor_copy(out=ob[:, 128 * m:128 * (m + 1)], in_=ptr[:])
                nc.sync.dma_start(out=o2_stage[128 * kt:128 * (kt + 1), :], in_=ob[:])
                if debug:
                    nc.sync.dma_start(out=d["t_o2"][128 * kt:128 * (kt + 1), :], in_=ob[:])

        o2_full = dram.tile([E * CAP, C], BF16, addr_space="Shared")
        nc.gpsimd.collective_compute(
            "AllGather", OP.bypass, replica_groups=[list(range(N_CORES))],
            ins=[o2_stage[:].opt()], outs=[o2_full[:].opt()])

        # ---- P12: final gather + gate + residual ----
        with tc.tile_pool(name="fsb", bufs=3) as fsb:
            for t in range(NHT):
                og = fsb.tile([128, C], BF16, name="og")
                nc.gpsimd.indirect_dma_start(
                    out=og[:], out_offset=None,
                    in_=o2_full[:],
                    in_offset=bass.IndirectOffsetOnAxis(ap=sclip_t[t][:, 0:1], axis=0))
                sg = fsb.tile([128, C], F32, name="sg")
                nc.vector.tensor_scalar(out=sg[:], in0=og[:], scalar1=gate_t[t][:],
                                        scalar2=None, op0=OP.mult)
                ot = fsb.tile([128, C], F32, name="ot")
                nc.vector.tensor_tensor(out=ot[:], in0=sg[:], in1=x2_sb[t][:], op=OP.add)
                nc.sync.dma_start(out=d["out"][128 * t:128 * (t + 1), :], in_=ot[:])


# ---------------------------------------------------------------------------
# Host side
# ---------------------------------------------------------------------------

def _fold(W, A, B_):
    return (np.asarray(W, np.float64)
            + SCALE * (np.asarray(A, np.float64).T @ np.asarray(B_, np.float64).T)
            ).astype(np.float32)


def _rope_tables():
    inv = 1.0 / (10000.0 ** (np.arange(0, HD, 2, dtype=np.float64) / HD))
    ang = np.arange(T, dtype=np.float64)[:, None] * inv
    cos = np.cos(ang).astype(np.float32)
    sin = np.sin(ang).astype(np.float32)
    return np.tile(cos, (1, 4)), np.tile(sin, (1, 4)), cos, sin


def _vmask():
    v = np.zeros((1, 66), np.float32)
    v[0, 64] = 1.0
    return v


def make_in_maps(inputs):
    x = np.asarray(inputs["x"], np.float32)
    aid = int(np.asarray(inputs["adapter_id"]))
    Wq = _fold(inputs["Wq"], np.asarray(inputs["Aq"])[aid], np.asarray(inputs["Bq"])[aid])
    Wk = _fold(inputs["Wk"], np.asarray(inputs["Ak"])[aid], np.asarray(inputs["Bk"])[aid])
    Wv = _fold(inputs["Wv"], np.asarray(inputs["Av"])[aid], np.asarray(inputs["Bv"])[aid])
    Wo = _fold(inputs["Wo"], np.asarray(inputs["Ao"])[aid], np.asarray(inputs["Bo"])[aid])
    Wr = np.ascontiguousarray(np.asarray(inputs["Wr"], np.float32))
    br = np.asarray(inputs["br"], np.float32).reshape(1, E)
    W1 = [_fold(np.asarray(inputs["W1"])[e], np.asarray(inputs["A1"])[e, aid],
                np.asarray(inputs["B1"])[e, aid]) for e in range(E)]
    W2 = [_fold(np.asarray(inputs["W2"])[e], np.asarray(inputs["A2"])[e, aid],
                np.asarray(inputs["B2"])[e, aid]) for e in range(E)]
    ln1g = np.asarray(inputs["ln1_g"], np.float32)
    ln1b = np.asarray(inputs["ln1_b"], np.float32)
    ln2g = np.asarray(inputs["ln2_g"], np.float32)
    ln2b = np.asarray(inputs["ln2_b"], np.float32)
    ln1 = np.concatenate([ln1g.reshape(3, 128).T, ln1b.reshape(3, 128).T], 1)
    ln2 = np.concatenate([ln2g.reshape(3, 128).T, ln2b.reshape(3, 128).T], 1)
    ln2gb = np.stack([ln2g, ln2b])
    cosq, sinq, cosk, sink = _rope_tables()

    in_maps = []
    for c in range(N_CORES):
        b, g = c // 2, c % 2
        e = c // 2
        wqkv = np.concatenate([
            Wq[:, 192 * g:192 * (g + 1)],
            Wk[:, 48 * g:48 * (g + 1)],
            Wv[:, 48 * g:48 * (g + 1)],
        ], axis=1)
        in_maps.append({
            "x": np.ascontiguousarray(x[b]),
            "xh": np.ascontiguousarray(x[b, TH * g:TH * (g + 1)]),
            "wqkv": np.ascontiguousarray(wqkv),
            "wo": Wo,
            "wr": Wr,
            "br": br,
            "w1": W1[e].astype(ml_dtypes.bfloat16),
            "w2": W2[e].astype(ml_dtypes.bfloat16),
            "ln1": np.ascontiguousarray(ln1),
            "ln2": np.ascontiguousarray(ln2),
            "ln2gb": np.ascontiguousarray(ln2gb),
            "cosq": np.ascontiguousarray(cosq),
            "sinq": np.ascontiguousarray(sinq),
            "cosk": np.ascontiguousarray(cosk),
            "sink": np.ascontiguousarray(sink),
            "wbase": (np.arange(8) < c).astype(np.float32).reshape(8, 1),
            "vmask": _vmask(),
            "ones48": np.ones((1, 48), np.float32),
            "cb": np.full((128, 1), TH * c, np.int32),
            "orow": (384 * g + 128 * np.arange(3)[None, :]
                     + np.arange(128)[:, None]).astype(np.int32),
        })
    return in_maps


_CACHED = {}


def _get_nc(debug=False):
    key = bool(debug)
    if key not in _CACHED:
        _CACHED[key] = build(key)
    return _CACHED[key]


def assemble(results):
    full = np.concatenate([results[c]["out"] for c in range(N_CORES)], 0)
    out = full.reshape(B, T, C)
    aux = np.float32(results[0]["aux"][0, 0])
    return out, aux


def kernel(**inputs):
    nc, _ = _get_nc(False)
    in_maps = make_in_maps(inputs)
    res = bass_utils.run_bass_kernel_spmd(nc, in_maps, core_ids=list(range(N_CORES)))
    return assemble(res.results)
